# revision 21
# baseline (speedup 1.0000x reference)
"""Trainium2 Bass kernel: 4-layer decoder prefill (S=1024, H=2048, NH=16, HD=128,
FFN=5632, V=32000), tensor-parallel over 8 NeuronCores.

v2 (rewrite of the f32r baseline):
- All weights stream as float16 (half the HBM traffic); activations q/k/v/attn/
  swig/probabilities are float16; the residual stream, norms and PSUM stay f32.
- Causal skip: upper-triangle key blocks are never computed; the 4 distinct
  diagonal triangle masks live in SBUF (no per-layer mask DMA).
- Weights load via a few large 3D-AP DMAs (wqkv/wo resident per layer,
  w13/w2 streamed in big tiles); AR staging is coalesced to 1 DMA per 4
  h-chunks.  Weight DMAs issue on the SP ring, AR staging on the ACT ring.
- AllReduce replaces ReduceScatter+AllGather.
- Last layer computes only K/V for all tokens + everything else 2-wide for the
  final token; w2/w13 accumulate 2-wide outputs packed into single PSUM banks.
"""

import os
import sys

sys.path.insert(0, "/opt/trn_rl_repo")

import numpy as np

L = int(os.environ.get("KERNEL_DEV_L", "4"))
B, S, H, NH, HD = 1, 1024, 2048, 16, 128
V, P = 32000, 5632
NC = 8
FEAT = H // NC          # 256 q/k/v features per core (2 heads)
PC = P // NC            # 704 ffn rows per core
PCP = 768               # padded to 6*128
VC = V // NC            # 4000 vocab rows per core
KH = H // 128           # 16 H-chunks
KP = PCP // 128         # 6 ffn chunks
EPS = 1e-5
SCALE = float(np.sqrt(HD))
INV_SCALE = 1.0 / SCALE

_STATE = {}


def _build():
    import concourse.bass as bass
    import concourse.bacc as bacc
    from concourse import tile, mybir

    F32 = mybir.dt.float32
    F32R = mybir.dt.float32r
    F16 = mybir.dt.float16
    AF = mybir.ActivationFunctionType
    ALU = mybir.AluOpType
    ts = bass.ts

    nc = bacc.Bacc("TRN2", target_bir_lowering=False, debug=False, num_devices=NC)

    xT_h = nc.dram_tensor("xT", [2, 128, KH * 512], F32, kind="ExternalInput")
    tri_h = nc.dram_tensor("tri", [128, 4 * 512], F16, kind="ExternalInput")
    C_h = nc.dram_tensor("Cr", [128, S], F16, kind="ExternalInput")
    S_h = nc.dram_tensor("Sr", [128, S], F16, kind="ExternalInput")
    J_h = nc.dram_tensor("J", [128, 128], F16, kind="ExternalInput")
    id_h = nc.dram_tensor("ident", [128, 128], F16, kind="ExternalInput")
    n1w_h = nc.dram_tensor("n1w", [128, L * KH], F32, kind="ExternalInput")
    n2w_h = nc.dram_tensor("n2w", [128, L * KH], F32, kind="ExternalInput")
    fw_h = nc.dram_tensor("fw", [128, KH], F32, kind="ExternalInput")
    # all weights pre-swizzled on the host into their exact SBUF layouts so
    # every DMA is a contiguous full-partition-line read
    wqkv_h = nc.dram_tensor("wqkvT", [L, 128, KH * 3 * FEAT], F16,
                            kind="ExternalInput")
    woT_h = nc.dram_tensor("woT", [L, 128, 2 * H], F16, kind="ExternalInput")
    # [mg][hc-pair][p][2 x (384 g | 384 u)]
    w13_h = nc.dram_tensor("w13T", [L, 2, KH // 2, 128, 2 * 768], F16,
                           kind="ExternalInput")
    # [hcb][p][kc*512]
    w2T_h = nc.dram_tensor("w2T", [L, 4, 128, KP * 512], F16,
                           kind="ExternalInput")
    owT_h = nc.dram_tensor("owT", [16, 128, KH * 250], F16,
                           kind="ExternalInput")
    out_h = nc.dram_tensor("logits", [1, VC], F32, kind="ExternalOutput")

    from contextlib import ExitStack

    with tile.TileContext(nc) as tc, ExitStack() as _ctx:
        ec = _ctx.enter_context
        p_resid = ec(tc.tile_pool(name="resid", bufs=1))
        p_const = ec(tc.tile_pool(name="consts", bufs=1))
        p_wqkv = ec(tc.tile_pool(name="wqkv", bufs=1))
        p_wo = ec(tc.tile_pool(name="wo", bufs=1))
        p_w13 = ec(tc.tile_pool(name="w13", bufs=3))
        p_w2 = ec(tc.tile_pool(name="w2", bufs=2))
        p_ow = ec(tc.tile_pool(name="ow", bufs=2))
        p_qkv = ec(tc.tile_pool(name="qkv", bufs=1))
        p_vs = ec(tc.tile_pool(name="vs", bufs=1))
        p_attn = ec(tc.tile_pool(name="attn", bufs=1))
        p_swig = ec(tc.tile_pool(name="swig", bufs=6))
        p_stage = ec(tc.tile_pool(name="stage", bufs=2))
        p_rd = ec(tc.tile_pool(name="rd", bufs=2))
        p_ns = ec(tc.tile_pool(name="ns", bufs=2))
        p_pt = ec(tc.tile_pool(name="pt", bufs=3))
        p_gs = ec(tc.tile_pool(name="gs", bufs=2))
        p_row = ec(tc.tile_pool(name="row", bufs=2))
        p_row1 = ec(tc.tile_pool(name="row1", bufs=1))
        p_t = ec(tc.tile_pool(name="tsc", bufs=2))
        psum = ec(tc.tile_pool(name="psum", bufs=8, space="PSUM"))
        dram = ec(tc.tile_pool(name="dram", bufs=4, space="DRAM"))

        def r3(ap, p=128):
            """[N*p, c] dram/sbuf AP -> [p, N, c] with partition first."""
            return ap.rearrange("(n p) c -> p n c", p=p)

        def sb3(ap, c):
            return ap.rearrange("p (n c) -> p n c", c=c)

        # ---------------- constants + residual ----------------
        xT = p_resid.tile([128, KH * S], F32, tag="xT")
        xT3 = xT[:].rearrange("p (hc s) -> p hc s", s=S)
        nc.sync.dma_start(
            xT3[:, :, ts(0, 512)],
            xT_h.ap()[0].rearrange("p (hc c) -> p hc c", c=512))

        wq_s0 = p_wqkv.tile([128, KH * 3 * FEAT], F16, tag="wqkv", name="wq0")
        nc.sync.dma_start(wq_s0[:], wqkv_h.ap()[0])
        wo_s0 = p_wo.tile([128, 2 * H], F16, tag="wo", name="wo0")
        nc.sync.dma_start(wo_s0[:], woT_h.ap()[0])
        # 2MB warm-up AllReduce: absorbs the cold ncfw + RDH-path cost while
        # the prologue computes (payload is garbage weights, output unused)
        warm_in = dram.tile([128, 8000], F16, tag="arin", name="wrm_in")
        warm_out = dram.tile([128, 8000], F16, tag="arout",
                             addr_space="Shared", name="wrm_out")
        for i in range(2):
            nc.sync.dma_start(warm_in[:, ts(i, 4000)], owT_h.ap()[i])
        nc.gpsimd.collective_compute(
            "AllReduce", ALU.add, replica_groups=[list(range(NC))],
            ins=[warm_in[:].opt()], outs=[warm_out[:].opt()])
        nc.sync.dma_start(
            xT3[:, :, ts(1, 512)],
            xT_h.ap()[1].rearrange("p (hc c) -> p hc c", c=512))

        tri_s = p_const.tile([128, 4 * 512], F16, tag="tri")
        nc.sync.dma_start(tri_s[:], tri_h.ap())
        C_s = p_const.tile([128, S], F16, tag="C")
        nc.sync.dma_start(C_s[:], C_h.ap())
        S_s = p_const.tile([128, S], F16, tag="S")
        nc.sync.dma_start(S_s[:], S_h.ap())
        J_r = p_const.tile([128, 128], F16, tag="J")
        nc.sync.dma_start(J_r[:], J_h.ap())
        id_r = p_const.tile([128, 128], F16, tag="id")
        nc.sync.dma_start(id_r[:], id_h.ap())
        n1w = p_const.tile([128, L * KH], F32, tag="n1w")
        nc.sync.dma_start(n1w[:], n1w_h.ap())
        n2w = p_const.tile([128, L * KH], F32, tag="n2w")
        nc.sync.dma_start(n2w[:], n2w_h.ap())
        fw_s = p_const.tile([128, KH], F32, tag="fw")
        nc.sync.dma_start(fw_s[:], fw_h.ap())

        ones_f = p_const.tile([128, 1], F32, tag="o1f")
        nc.vector.memset(ones_f[:], 1.0)
        ones_col = p_const.tile([128, 1], F16, tag="o1")
        nc.vector.tensor_copy(ones_col[:], ones_f[:])
        ones_rf = p_const.tile([1, 128], F32, tag="orf")
        nc.vector.memset(ones_rf[:], 1.0)
        ones_row = p_const.tile([1, 128], F32R, tag="or")
        nc.vector.tensor_copy(ones_row[:], ones_rf[:])
        onc_r = p_const.tile([128, 1], F32R, tag="o1r")
        nc.vector.tensor_copy(onc_r[:], ones_f[:])
        eps_t = p_const.tile([1, 1], F32, tag="eps")
        nc.vector.memset(eps_t[:], EPS)
        eps_p = p_const.tile([128, 1], F32, tag="epsp")
        nc.vector.memset(eps_p[:], EPS)
        ones_mf = p_const.tile([128, 128], F32, tag="omf")
        nc.vector.memset(ones_mf[:], 1.0)
        ones_mat = p_const.tile([128, 128], F16, tag="om")
        nc.vector.tensor_copy(ones_mat[:], ones_mf[:])

        # ---------------- per-layer weight loads ----------------
        def load_wqkv(l_):
            w = p_wqkv.tile([128, KH * 3 * FEAT], F16, tag="wqkv", name=f"wq{l_}")
            nc.sync.dma_start(w[:], wqkv_h.ap()[l_])
            return w

        def load_wo(l_):
            w = p_wo.tile([128, 2 * H], F16, tag="wo", name=f"wo{l_}")
            nc.sync.dma_start(w[:], woT_h.ap()[l_])
            return w

        def norm_half(w_tile, l_, tk, ar_out=None):
            """1/rms for tokens [tk*512, tk*512+512) broadcast into PSUM
            [128,512]; returns the psum tile (keep alive while using).
            If ar_out is given, the residual add for this half is fused in
            front of the square pass chunk by chunk."""
            ssum = psum.tile([1, 512], F32, tag="ps", name="ssum")
            for hc in range(KH):
                if ar_out is not None and hc % 2 == 0:
                    hcb = hc // 2
                    rd = p_rd.tile([128, 2 * 512], F16, tag="rd", name="rd")
                    nc.scalar.dma_start(rd[:], ar_out[:, ts(hcb, 1024)])
                    nc.vector.tensor_add(
                        xT3[:, hc:hc + 2, ts(tk, 512)],
                        xT3[:, hc:hc + 2, ts(tk, 512)],
                        rd[:].rearrange("p (j c) -> p j c", c=512))
                sq = p_pt.tile([128, 512], F16, tag="pt", name="sq")
                if hc % 2 == 0:
                    nc.vector.tensor_mul(sq[:], xT3[:, hc, ts(tk, 512)],
                                         xT3[:, hc, ts(tk, 512)])
                else:
                    nc.scalar.activation(sq[:], xT3[:, hc, ts(tk, 512)],
                                         AF.Square)
                nc.tensor.matmul(ssum[:], ones_col[:], sq[:],
                                 start=(hc == 0), stop=(hc == KH - 1))
            rms = p_row1.tile([1, 512], F32, tag="rms")
            nc.scalar.activation(rms[:], ssum[:], AF.Sqrt,
                                 bias=eps_t[:], scale=1.0 / H)
            inv = p_row1.tile([1, 512], F32, tag="inv")
            nc.vector.reciprocal_approx_fast(out=inv[:], in_=rms[:])
            invr = p_row1.tile([1, 512], F32R, tag="invr")
            with nc.allow_low_precision(reason="f32r cast of 1/rms"):
                nc.vector.tensor_copy(invr[:], inv[:])
            bc_ps = psum.tile([128, 512], F32, tag="ps", name="bcps")
            nc.tensor.matmul(bc_ps[:], ones_row[:], invr[:], start=True,
                             stop=True)
            return bc_ps

        def qkv_half(l_, tk, wq_s, q_s, k_s, vT_s, q_only_last=False,
                     ar_out=None):
            """QKV for token half tk of layer l_.  If q_only_last, compute q
            just for the final 2 tokens (last layer)."""
            bc = norm_half(n1w, l_, tk, ar_out=ar_out)
            kp = [psum.tile([128, 512], F32, tag="ps", name=f"kp{i}")
                  for i in range(2)]
            vp = [psum.tile([128, 512], F32, tag="ps", name=f"vp{i}")
                  for i in range(2)]
            if not q_only_last:
                qp = [psum.tile([128, 512], F32, tag="ps", name=f"qp{i}")
                      for i in range(2)]
            elif tk == 1:
                qp = [psum.tile([128, 2], F32, tag="ps", name=f"q2p{i}")
                      for i in range(2)]
            else:
                qp = None
            for hc in range(KH):
                xn = p_ns.tile([128, 512], F16, tag="ns", name="xn")
                nc.vector.scalar_tensor_tensor(
                    xn[:], xT3[:, hc, ts(tk, 512)],
                    n1w[:, l_ * KH + hc: l_ * KH + hc + 1],
                    bc[:], op0=ALU.mult, op1=ALU.mult)
                st, sp = (hc == 0), (hc == KH - 1)
                woff = hc * 3 * FEAT
                for mt in range(2):
                    nc.tensor.matmul(
                        kp[mt][:], wq_s[:, woff + 256 + mt * 128:
                                        woff + 384 + mt * 128],
                        xn[:], start=st, stop=sp)
                    nc.tensor.matmul(
                        vp[mt][:], wq_s[:, woff + 512 + mt * 128:
                                        woff + 640 + mt * 128],
                        xn[:], start=st, stop=sp)
                    if qp is not None and not q_only_last:
                        nc.tensor.matmul(
                            qp[mt][:], wq_s[:, woff + mt * 128: woff + (mt + 1) * 128],
                            xn[:], start=st, stop=sp)
                    elif qp is not None:
                        nc.tensor.matmul(
                            qp[mt][:],
                            wq_s[:, woff + mt * 128: woff + (mt + 1) * 128],
                            xn[:, 510:512], start=st, stop=sp)
            for mt in range(2):
                off = mt * S + tk * 512
                nc.scalar.activation(k_s[:, off:off + 512], kp[mt][:], AF.Copy)
                nc.scalar.activation(vT_s[:, off:off + 512], vp[mt][:], AF.Copy)
                if qp is not None and not q_only_last:
                    nc.scalar.activation(q_s[:, off:off + 512], qp[mt][:],
                                         AF.Copy)
                elif qp is not None:
                    nc.scalar.activation(q_s[:, mt * S + S - 2: mt * S + S],
                                         qp[mt][:], AF.Copy)

        def rope(t_s, cols=None):
            """In-place interleaved rope on [128, 2*S] f16 (cols: list of
            (col_off, width, cs_off) to process; default full)."""
            if cols is None:
                cols = [(mt * S + n * 512, 512, n * 512)
                        for mt in range(2) for n in range(2)]
            for off, w, cs in cols:
                j_ps = psum.tile([128, 512], F32, tag="ps", name="jps")
                nc.tensor.matmul(j_ps[:, :w], J_r[:], t_s[:, off:off + w],
                                 start=True, stop=True)
                tmp = p_t.tile([128, 512], F16, tag="rt", name="rtmp")
                nc.vector.tensor_mul(tmp[:, :w], C_s[:, cs:cs + w],
                                     t_s[:, off:off + w])
                nc.vector.tensor_mul(t_s[:, off:off + w], j_ps[:, :w],
                                     S_s[:, cs:cs + w])
                nc.vector.tensor_add(t_s[:, off:off + w], t_s[:, off:off + w],
                                     tmp[:, :w])

        def v_transpose(vT_s, v_s, tk):
            for mt in range(2):
                for tb in range(tk * 4, tk * 4 + 4):
                    tp = psum.tile([128, 128], F16, tag="ps", name="tp")
                    nc.tensor.transpose(
                        tp[:], vT_s[:, mt * S + tb * 128: mt * S + tb * 128 + 128],
                        id_r[:])
                    nc.scalar.activation(
                        v_s[:, tb * FEAT + mt * 128: tb * FEAT + mt * 128 + 128],
                        tp[:], AF.Copy)

        def rope_half(t_s, tk):
            rope(t_s, cols=[(mt * S + tk * 512, 512, tk * 512)
                            for mt in range(2)])

        def qkv_tail(l_, tk, wq, q_s, k_s, vT_s, v_s, ar_out=None):
            qkv_half(l_, tk, wq, q_s, k_s, vT_s,
                     q_only_last=(l_ == L - 1), ar_out=ar_out)
            if l_ == L - 1:
                if tk == 1:
                    rope(q_s, cols=[(mt * S + S - 2, 2, S - 2)
                                    for mt in range(2)])
            else:
                rope_half(q_s, tk)
            rope_half(k_s, tk)
            v_transpose(vT_s, v_s, tk)

        def attention(tk, h, q_s, k_s, v_s, attn_s):
            at_ps = psum.tile([128, 512], F32, tag="ps", name="atp")
            rs_ps = psum.tile([1, 512], F32, tag="ps", name="rsp")
            nkc = 4 * (tk + 1)
            for kc in range(nkc):
                sc_ps = psum.tile([128, 512], F32, tag="ps", name="scp")
                nc.tensor.matmul(
                    sc_ps[:], k_s[:, h * S + kc * 128: h * S + kc * 128 + 128],
                    q_s[:, h * S + tk * 512: h * S + tk * 512 + 512],
                    start=True, stop=True)
                pt = p_pt.tile([128, 512], F16, tag="pt", name="ptl")
                d = kc - 4 * tk
                if d >= 0:
                    nc.vector.scalar_tensor_tensor(
                        sc_ps[:], sc_ps[:], INV_SCALE, tri_s[:, ts(d, 512)],
                        op0=ALU.mult, op1=ALU.add)
                    nc.scalar.activation(pt[:], sc_ps[:], AF.Exp)
                else:
                    nc.scalar.activation(pt[:], sc_ps[:], AF.Exp,
                                         scale=INV_SCALE)
                st, sp = (kc == 0), (kc == nkc - 1)
                nc.tensor.matmul(
                    at_ps[:], v_s[:, kc * FEAT + h * 128: kc * FEAT + h * 128 + 128],
                    pt[:], start=st, stop=sp)
                nc.tensor.matmul(rs_ps[:], ones_col[:], pt[:], start=st, stop=sp)
            inv = p_row1.tile([1, 512], F32, tag="inv")
            nc.vector.reciprocal_approx_fast(out=inv[:], in_=rs_ps[:])
            invr = p_row1.tile([1, 512], F32R, tag="invr")
            with nc.allow_low_precision(reason="f32r cast of 1/sum"):
                nc.vector.tensor_copy(invr[:], inv[:])
            ib_ps = psum.tile([128, 512], F32, tag="ps", name="ibp")
            nc.tensor.matmul(ib_ps[:], ones_row[:], invr[:], start=True,
                             stop=True)
            ib_s = p_gs.tile([128, 512], F16, tag="ib", name="ibs")
            nc.scalar.activation(ib_s[:], ib_ps[:], AF.Copy)
            nc.vector.tensor_mul(
                attn_s[:, h * S + tk * 512: h * S + tk * 512 + 512],
                at_ps[:], ib_s[:])

        def wo_stage(l_, tk, wo_s, attn_s):
            """wo projection for half tk -> staged f16 AR input in DRAM.
            AR buffers are p-major [128, KH*512] (layout-consistent across
            cores, so the elementwise AllReduce is still correct)."""
            ar_in = dram.tile([128, KH * 512], F16, tag="arin", name="arin")
            ar_out = dram.tile([128, KH * 512], F16, tag="arout",
                               addr_space="Shared", name="arout")
            for hcb in range(8):
                st_t = p_stage.tile([128, 2 * 512], F16, tag="st", name="st")
                for hh in range(2):
                    hc = hcb * 2 + hh
                    po = psum.tile([128, 512], F32, tag="ps", name="po")
                    for fc in range(2):
                        nc.tensor.matmul(
                            po[:], wo_s[:, fc * H + hc * 128: fc * H + hc * 128 + 128],
                            attn_s[:, fc * S + tk * 512: fc * S + tk * 512 + 512],
                            start=(fc == 0), stop=(fc == 1))
                    nc.scalar.activation(st_t[:, ts(hh, 512)], po[:], AF.Copy)
                nc.scalar.dma_start(ar_in[:, ts(hcb, 1024)], st_t[:])
            nc.gpsimd.collective_compute(
                "AllReduce", ALU.add, replica_groups=[list(range(NC))],
                ins=[ar_in[:].opt()], outs=[ar_out[:].opt()])
            return ar_out

        def resid_add(tk, ar_out):
            for hcb in range(8):
                rd = p_rd.tile([128, 2 * 512], F16, tag="rd", name="rd")
                nc.scalar.dma_start(rd[:], ar_out[:, ts(hcb, 1024)])
                nc.vector.tensor_add(
                    xT3[:, hcb * 2:hcb * 2 + 2, ts(tk, 512)],
                    xT3[:, hcb * 2:hcb * 2 + 2, ts(tk, 512)],
                    rd[:].rearrange("p (j c) -> p j c", c=512))

        def ffn_half(l_, tk, swig, ar_out=None):
            """norm2 + w1/w3 + swiglu for half tk (into swig[kc] f16 tiles)."""
            bc2 = norm_half(n2w, l_, tk, ar_out=ar_out)
            for mg in range(2):
                mts = [0, 1, 2] if mg == 0 else [3, 4, 5]
                gp = {mt: psum.tile([128, 512], F32, tag="ps", name=f"gp{mt}")
                      for mt in mts}
                up = {mt: psum.tile([128, 512], F32, tag="ps", name=f"up{mt}")
                      for mt in mts}
                for hp in range(KH // 2):
                    wt = p_w13.tile([128, 1536], F16, tag="w13", name="wt13")
                    nc.sync.dma_start(wt[:], w13_h.ap()[l_, mg, hp])
                    for sub in range(2):
                        hc = hp * 2 + sub
                        hn = p_ns.tile([128, 512], F16, tag="ns", name="hn")
                        nc.vector.scalar_tensor_tensor(
                            hn[:], xT3[:, hc, ts(tk, 512)],
                            n2w[:, l_ * KH + hc: l_ * KH + hc + 1],
                            bc2[:], op0=ALU.mult, op1=ALU.mult)
                        st, sp = (hc == 0), (hc == KH - 1)
                        for i, mt in enumerate(mts):
                            nc.tensor.matmul(
                                gp[mt][:], wt[:, sub * 768 + i * 128:
                                              sub * 768 + (i + 1) * 128],
                                hn[:], start=st, stop=sp)
                            nc.tensor.matmul(
                                up[mt][:], wt[:, sub * 768 + 384 + i * 128:
                                              sub * 768 + 384 + (i + 1) * 128],
                                hn[:], start=st, stop=sp)
                for mt in mts:
                    gs = p_gs.tile([128, 512], F16, tag="gs", name="gs")
                    nc.scalar.activation(gs[:], gp[mt][:], AF.Silu)
                    nc.vector.tensor_mul(swig[mt][:, ts(tk, 512)],
                                         up[mt][:], gs[:])

        def w2_stage(l_, tk, swig):
            ar_in = dram.tile([128, KH * 512], F16, tag="arin", name="ar2in")
            ar_out = dram.tile([128, KH * 512], F16, tag="arout",
                               addr_space="Shared", name="ar2out")
            for hcb in range(4):
                p2 = [psum.tile([128, 512], F32, tag="ps", name=f"p2{i}")
                      for i in range(4)]
                wt = p_w2.tile([128, KP * 512], F16, tag="w2", name="w2t")
                nc.sync.dma_start(wt[:], w2T_h.ap()[l_, hcb])
                for kc in range(KP):
                    for hh in range(4):
                        nc.tensor.matmul(
                            p2[hh][:], wt[:, kc * 512 + hh * 128:
                                          kc * 512 + (hh + 1) * 128],
                            swig[kc][:, ts(tk, 512)],
                            start=(kc == 0), stop=(kc == KP - 1))
                for sg in range(2):
                    st_t = p_stage.tile([128, 2 * 512], F16, tag="st",
                                        name="st2")
                    for hh in range(2):
                        nc.scalar.activation(st_t[:, ts(hh, 512)],
                                             p2[sg * 2 + hh][:], AF.Copy)
                    nc.scalar.dma_start(
                        ar_in[:, ts(hcb * 2 + sg, 1024)], st_t[:])
            nc.gpsimd.collective_compute(
                "AllReduce", ALU.add, replica_groups=[list(range(NC))],
                ins=[ar_in[:].opt()], outs=[ar_out[:].opt()])
            return ar_out

        # ================= prologue =================
        wq_s = wq_s0
        wo_s = wo_s0
        cur_q = p_qkv.tile([128, 2 * S], F16, tag="q", name="q0")
        cur_k = p_qkv.tile([128, 2 * S], F16, tag="k", name="k0")
        cur_vT = p_qkv.tile([128, 2 * S], F16, tag="vT", name="vT0")
        cur_v = p_vs.tile([128, 8 * FEAT], F16, tag="v", name="vs0")
        qkv_tail(0, 0, wq_s, cur_q, cur_k, cur_vT, cur_v)
        attn_s = p_attn.tile([128, 2 * S], F16, tag="attn", name="attn0")
        for h in range(2):
            attention(0, h, cur_q, cur_k, cur_v, attn_s)
        ar1_0 = wo_stage(0, 0, wo_s, attn_s)
        qkv_tail(0, 1, wq_s, cur_q, cur_k, cur_vT, cur_v)
        for h in range(2):
            attention(1, h, cur_q, cur_k, cur_v, attn_s)
        ar1_1 = wo_stage(0, 1, wo_s, attn_s)

        for l in range(L - 1):
            nxt_wq = load_wqkv(l + 1)
            nxt_wo = load_wo(l + 1)
            swig = [p_swig.tile([128, S], F16, tag="sw", name=f"swig{i}")
                    for i in range(KP)]
            ffn_half(l, 0, swig, ar_out=ar1_0)
            ar2_0 = w2_stage(l, 0, swig)
            ffn_half(l, 1, swig, ar_out=ar1_1)
            ar2_1 = w2_stage(l, 1, swig)

            nxt_q = p_qkv.tile([128, 2 * S], F16, tag="q", name="qn")
            nxt_k = p_qkv.tile([128, 2 * S], F16, tag="k", name="kn")
            nxt_vT = p_qkv.tile([128, 2 * S], F16, tag="vT", name="vTn")
            nxt_v = p_vs.tile([128, 8 * FEAT], F16, tag="v", name="vsn")
            qkv_tail(l + 1, 0, nxt_wq, nxt_q, nxt_k, nxt_vT, nxt_v,
                     ar_out=ar2_0)
            if l + 1 < L - 1:
                attn_s = p_attn.tile([128, 2 * S], F16, tag="attn",
                                     name="attn")
                for h in range(2):
                    attention(0, h, nxt_q, nxt_k, nxt_v, attn_s)
                ar1_0 = wo_stage(l + 1, 0, nxt_wo, attn_s)
                qkv_tail(l + 1, 1, nxt_wq, nxt_q, nxt_k, nxt_vT, nxt_v,
                         ar_out=ar2_1)
                for h in range(2):
                    attention(1, h, nxt_q, nxt_k, nxt_v, attn_s)
                ar1_1 = wo_stage(l + 1, 1, nxt_wo, attn_s)
            else:
                qkv_tail(l + 1, 1, nxt_wq, nxt_q, nxt_k, nxt_vT, nxt_v,
                         ar_out=ar2_1)
            cur_q, cur_k, cur_vT, cur_v = nxt_q, nxt_k, nxt_vT, nxt_v
            wq_s, wo_s = nxt_wq, nxt_wo

        # ================= last layer =================
        l = L - 1
        q_s, k_s, vT_s, v_s = cur_q, cur_k, cur_vT, cur_v
        if True:
            if True:
                # ---- single-query attention (2-wide) ----
                attn_s = p_attn.tile([128, 2 * S], F16, tag="attn", name="attnL")
                at1 = [psum.tile([128, 2], F32, tag="ps", name=f"at1{h}")
                       for h in range(2)]
                rs1 = [psum.tile([128, 2], F32, tag="ps", name=f"rs1{h}")
                       for h in range(2)]
                for kc in range(8):
                    for h in range(2):
                        sc1 = psum.tile([128, 2], F32, tag="ps", name="sc1")
                        nc.tensor.matmul(
                            sc1[:],
                            k_s[:, h * S + kc * 128: h * S + kc * 128 + 128],
                            q_s[:, h * S + S - 2: h * S + S],
                            start=True, stop=True)
                        pt1 = p_row.tile([128, 2], F16, tag="sm", name="pt1")
                        if kc == 7:
                            nc.vector.scalar_tensor_tensor(
                                sc1[:], sc1[:], INV_SCALE,
                                tri_s[:, 3 * 512 + 510: 3 * 512 + 512],
                                op0=ALU.mult, op1=ALU.add)
                            nc.scalar.activation(pt1[:], sc1[:], AF.Exp)
                        else:
                            nc.scalar.activation(pt1[:], sc1[:], AF.Exp,
                                                 scale=INV_SCALE)
                        st, sp = (kc == 0), (kc == 7)
                        nc.tensor.matmul(
                            at1[h][:],
                            v_s[:, kc * FEAT + h * 128: kc * FEAT + h * 128 + 128],
                            pt1[:], start=st, stop=sp)
                        nc.tensor.matmul(rs1[h][:], ones_mat[:], pt1[:],
                                         start=st, stop=sp)
                for h in range(2):
                    inva = p_row.tile([128, 2], F32, tag="sm", name="inva")
                    nc.vector.reciprocal(inva[:], rs1[h][:])
                    nc.vector.tensor_mul(
                        attn_s[:, h * S + S - 2: h * S + S], at1[h][:],
                        inva[:])

                # wo (resident) -> [H,2] AllReduce -> residual add
                ar_in = dram.tile([128, KH * 2], F16, tag="arinL",
                                  name="arinL")
                ar_out = dram.tile([128, KH * 2], F16, tag="aroutL",
                                   addr_space="Shared", name="aroutL")
                stL = p_row.tile([128, KH * 2], F16, tag="stL", name="stL")
                for hc in range(KH):
                    poL = psum.tile([128, 2], F32, tag="ps", name="poL")
                    for fc in range(2):
                        nc.tensor.matmul(
                            poL[:], wo_s[:, fc * H + hc * 128: fc * H + hc * 128 + 128],
                            attn_s[:, fc * S + S - 2: fc * S + S],
                            start=(fc == 0), stop=(fc == 1))
                    nc.scalar.activation(stL[:, ts(hc, 2)], poL[:], AF.Copy)
                nc.scalar.dma_start(ar_in[:], stL[:])
                nc.gpsimd.collective_compute(
                    "AllReduce", ALU.add, replica_groups=[list(range(NC))],
                    ins=[ar_in[:].opt()], outs=[ar_out[:].opt()])
                rdL = p_row.tile([128, KH * 2], F16, tag="stL", name="rdL")
                nc.scalar.dma_start(rdL[:], ar_out[:])
                nc.vector.tensor_add(
                    xT3[:, :, S - 2:S], xT3[:, :, S - 2:S],
                    rdL[:].rearrange("p (j c) -> p j c", c=2))

                # ---- norm2 + FFN on the last 2 tokens ----
                sqL = p_row.tile([128, 2 * KH], F16, tag="sql2")
                for hc in range(KH):
                    nc.vector.tensor_mul(sqL[:, 2 * hc:2 * hc + 2],
                                         xT3[:, hc, S - 2:S],
                                         xT3[:, hc, S - 2:S])
                ssL = psum.tile([128, 2 * KH], F32, tag="ps", name="ssL")
                nc.tensor.matmul(ssL[:], ones_mat[:], sqL[:],
                                 start=True, stop=True)
                ssr = p_row.tile([128, 2], F32, tag="ssr")
                nc.vector.reduce_sum(
                    ssr[:], ssL[:].rearrange("p (c two) -> p two c", two=2),
                    axis=mybir.AxisListType.X)
                rmsL = p_row.tile([128, 2], F32, tag="rmsL")
                nc.scalar.activation(rmsL[:], ssr[:], AF.Sqrt,
                                     bias=eps_p[:], scale=1.0 / H)
                invL = p_row.tile([128, 2], F32, tag="invLc")
                nc.vector.reciprocal(invL[:], rmsL[:])
                hnL = p_row.tile([128, 2 * KH], F16, tag="hnL")
                tnL = p_row.tile([128, 2], F32, tag="tnL")
                for hc in range(KH):
                    nc.vector.tensor_scalar_mul(
                        tnL[:], xT3[:, hc, S - 2:S],
                        n2w[:, l * KH + hc: l * KH + hc + 1])
                    nc.vector.tensor_mul(hnL[:, 2 * hc:2 * hc + 2],
                                         tnL[:], invL[:])
                # w1/w3 flipped: stationary 2-wide hnL, moving w13 cols
                gT = [psum.tile([2, 384], F32, tag="ps", name=f"gT{i}")
                      for i in range(2)]
                uT = [psum.tile([2, 384], F32, tag="ps", name=f"uT{i}")
                      for i in range(2)]
                for mg in range(2):
                    for hp in range(KH // 2):
                        wt = p_w13.tile([128, 1536], F16, tag="w13",
                                        name="wt13L")
                        nc.sync.dma_start(wt[:], w13_h.ap()[l, mg, hp])
                        for sub in range(2):
                            hc = hp * 2 + sub
                            st_, sp_ = (hc == 0), (hc == KH - 1)
                            nc.tensor.matmul(
                                gT[mg][:], hnL[:, 2 * hc:2 * hc + 2],
                                wt[:, sub * 768: sub * 768 + 384],
                                start=st_, stop=sp_)
                            nc.tensor.matmul(
                                uT[mg][:], hnL[:, 2 * hc:2 * hc + 2],
                                wt[:, sub * 768 + 384: sub * 768 + 768],
                                start=st_, stop=sp_)
                gsT = p_row1.tile([2, 768], F16, tag="gsT")
                swT = p_row1.tile([2, 768], F16, tag="swT")
                for mg in range(2):
                    nc.scalar.activation(gsT[:, ts(mg, 384)], gT[mg][:],
                                         AF.Silu)
                    nc.vector.tensor_mul(swT[:, ts(mg, 384)], uT[mg][:],
                                         gsT[:, ts(mg, 384)])
                # transpose swigT [2,768] -> swigL [128, 12] (kc-major 2-wide)
                swigL = p_row.tile([128, 12], F16, tag="swL")
                for kc in range(KP):
                    tpL = psum.tile([128, 2], F16, tag="ps", name="tpL")
                    nc.tensor.transpose(tpL[:], swT[:, ts(kc, 128)],
                                        id_r[:2, :2])
                    nc.scalar.activation(swigL[:, ts(kc, 2)], tpL[:], AF.Copy)
                # w2 flipped: stationary 2-wide swigL, moving w2 hcb tiles
                ar2_in = dram.tile([128, KH * 2], F16, tag="arinL",
                                   name="ar2inL")
                ar2_out = dram.tile([128, KH * 2], F16, tag="aroutL",
                                    addr_space="Shared", name="ar2outL")
                p2T = [psum.tile([2, 512], F32, tag="ps", name=f"p2T{i}")
                       for i in range(4)]
                for hcb in range(4):
                    wt = p_w2.tile([128, KP * 512], F16, tag="w2", name="w2tL")
                    nc.sync.dma_start(wt[:], w2T_h.ap()[l, hcb])
                    for kc in range(KP):
                        nc.tensor.matmul(
                            p2T[hcb][:], swigL[:, ts(kc, 2)],
                            wt[:, ts(kc, 512)],
                            start=(kc == 0), stop=(kc == KP - 1))
                st2L = p_row.tile([128, KH * 2], F16, tag="stL", name="st2L")
                for hcb in range(4):
                    p2c = p_gs.tile([2, 512], F16, tag="p2c", name="p2c")
                    nc.scalar.activation(p2c[:], p2T[hcb][:], AF.Copy)
                    for hh in range(4):
                        hc = hcb * 4 + hh
                        tp2 = psum.tile([128, 2], F16, tag="ps", name="tp2")
                        nc.tensor.transpose(tp2[:], p2c[:, ts(hh, 128)],
                                            id_r[:2, :2])
                        nc.scalar.activation(st2L[:, ts(hc, 2)], tp2[:],
                                             AF.Copy)
                nc.scalar.dma_start(ar2_in[:], st2L[:])
                nc.gpsimd.collective_compute(
                    "AllReduce", ALU.add, replica_groups=[list(range(NC))],
                    ins=[ar2_in[:].opt()], outs=[ar2_out[:].opt()])
                rd2L = p_row.tile([128, KH * 2], F16, tag="stL", name="rd2L")
                nc.scalar.dma_start(rd2L[:], ar2_out[:])
                nc.vector.tensor_add(
                    xT3[:, :, S - 2:S], xT3[:, :, S - 2:S],
                    rd2L[:].rearrange("p (j c) -> p j c", c=2))
        # ======== final norm (last token only) + logits ========
        sq_l = p_row.tile([128, KH], F32R, tag="sql")
        for hc in range(KH):
            nc.vector.tensor_mul(sq_l[:, hc:hc + 1], xT3[:, hc, S - 1:S],
                                 xT3[:, hc, S - 1:S])
        sl_ps = psum.tile([1, KH], F32, tag="ps", name="slps")
        nc.tensor.matmul(sl_ps[:], onc_r[:], sq_l[:], start=True, stop=True)
        ssc = p_row.tile([1, 1], F32, tag="ssc")
        nc.vector.reduce_sum(ssc[:], sl_ps[:], axis=mybir.AxisListType.X)
        rms_l = p_row.tile([1, 1], F32, tag="rmsl")
        nc.scalar.activation(rms_l[:], ssc[:], AF.Sqrt, bias=eps_t[:],
                             scale=1.0 / H)
        inv_l = p_row.tile([1, 1], F32, tag="invl")
        nc.vector.reciprocal(inv_l[:], rms_l[:])
        xnl = p_row.tile([128, KH], F16, tag="xnl")
        for hc in range(KH):
            nc.vector.tensor_mul(xnl[:, hc:hc + 1], xT3[:, hc, S - 1:S],
                                 fw_s[:, hc:hc + 1])
        NCHUNK = 16
        CW = VC // NCHUNK  # 250
        for n in range(NCHUNK):
            ow_t = p_ow.tile([128, KH * CW], F16, tag="ow", name="owt")
            nc.sync.dma_start(ow_t[:], owT_h.ap()[n])
            lg_ps = psum.tile([1, CW], F32, tag="ps", name="lgps")
            for hc in range(KH):
                nc.tensor.matmul(lg_ps[:], xnl[:, hc: hc + 1],
                                 ow_t[:, ts(hc, CW)],
                                 start=(hc == 0), stop=(hc == KH - 1))
            lg = p_row1.tile([1, CW], F32, tag="lg")
            nc.scalar.activation(lg[:], lg_ps[:], AF.Copy, scale=inv_l[:])
            nc.sync.dma_start(out_h.ap()[:, ts(n, CW)], lg[:])

    nc.compile()
    return nc


def _shard(inputs):
    f16 = np.float16
    x = np.asarray(inputs["x"], np.float32)
    mask = np.asarray(inputs["attn_mask"], np.float32)
    cos = np.asarray(inputs["cos"], np.float32).reshape(S, HD // 2)
    sin = np.asarray(inputs["sin"], np.float32).reshape(S, HD // 2)
    n1 = np.asarray(inputs["norm1_w"], np.float32)[:L]
    n2 = np.asarray(inputs["norm2_w"], np.float32)[:L]
    fw = np.asarray(inputs["final_norm_w"], np.float32)
    wq = np.asarray(inputs["wq"], np.float32)[:L]
    wk = np.asarray(inputs["wk"], np.float32)[:L]
    wv = np.asarray(inputs["wv"], np.float32)[:L]
    wo = np.asarray(inputs["wo"], np.float32)[:L]
    w1 = np.asarray(inputs["w1"], np.float32)[:L]
    w3 = np.asarray(inputs["w3"], np.float32)[:L]
    w2 = np.asarray(inputs["w2"], np.float32)[:L]
    ow = np.asarray(inputs["out_w"], np.float32)

    def pmajor(a):
        """[N*128, c] -> [128, N*c] (SBUF layout: partition-major)."""
        n = a.shape[0] // 128
        return np.ascontiguousarray(
            a.reshape(n, 128, a.shape[1]).transpose(1, 0, 2)
            .reshape(128, n * a.shape[1]))

    xTf = x[0].T  # [H, S]
    xT = np.stack([pmajor(np.ascontiguousarray(xTf[:, tk * 512:(tk + 1) * 512]))
                   for tk in range(2)])

    maskT = mask[0].T  # [key, query]
    tri = np.ascontiguousarray(
        np.concatenate([maskT[d * 128:(d + 1) * 128, 0:512] for d in range(4)],
                       axis=1))
    tri = np.clip(tri, -30000.0, 0.0).astype(f16)
    C = np.empty((128, S), np.float32)
    C[0::2] = cos.T
    C[1::2] = cos.T
    Sm = np.empty((128, S), np.float32)
    Sm[0::2] = -sin.T
    Sm[1::2] = sin.T
    C = C.astype(f16)
    Sm = Sm.astype(f16)
    J = np.zeros((128, 128), f16)
    idx = np.arange(0, 128, 2)
    J[idx, idx + 1] = 1.0
    J[idx + 1, idx] = 1.0
    ident = np.eye(128, dtype=f16)
    n1w = np.ascontiguousarray(
        n1.reshape(L, KH, 128).transpose(2, 0, 1).reshape(128, L * KH))
    n2w = np.ascontiguousarray(
        n2.reshape(L, KH, 128).transpose(2, 0, 1).reshape(128, L * KH))
    fwh = np.ascontiguousarray(fw.reshape(KH, 128).T)

    common = dict(xT=xT, tri=tri, Cr=C, Sr=Sm, J=J, ident=ident,
                  n1w=n1w, n2w=n2w, fw=fwh)
    in_maps = []
    for c in range(NC):
        fs = slice(c * FEAT, (c + 1) * FEAT)
        ps = slice(c * PC, (c + 1) * PC)
        vs = slice(c * VC, (c + 1) * VC)
        m = dict(common)
        wqT = wq[:, fs, :].transpose(0, 2, 1)
        wkT = wk[:, fs, :].transpose(0, 2, 1)
        wvT = wv[:, fs, :].transpose(0, 2, 1)
        wqkv = np.concatenate([wqT, wkT, wvT], axis=2)  # [L, H, 768]
        m["wqkvT"] = np.stack([pmajor(wqkv[l]) for l in range(L)]).astype(f16)
        woT = wo[:, :, fs].transpose(0, 2, 1)  # [L, FEAT, H]
        m["woT"] = np.stack([pmajor(woT[l]) for l in range(L)]).astype(f16)
        w1T = np.zeros((L, H, PCP), np.float32)
        w3T = np.zeros((L, H, PCP), np.float32)
        w1T[:, :, :PC] = w1[:, ps, :].transpose(0, 2, 1)
        w3T[:, :, :PC] = w3[:, ps, :].transpose(0, 2, 1)
        # [L, 2(mg), 8(hc-pair), 128, 2 x (384 g | 384 u)]
        w13 = np.empty((L, 2, KH // 2, 128, 1536), np.float32)
        for mg in range(2):
            g = w1T[:, :, mg * 384:(mg + 1) * 384]
            u = w3T[:, :, mg * 384:(mg + 1) * 384]
            gu = np.concatenate([g, u], axis=2)  # [L, H, 768]
            w13[:, mg] = gu.reshape(L, KH // 2, 2 * 128, 768) \
                           .reshape(L, KH // 2, 2, 128, 768) \
                           .transpose(0, 1, 3, 2, 4).reshape(L, KH // 2, 128, 1536)
        m["w13T"] = w13.astype(f16)
        w2p = np.zeros((L, PCP, H), np.float32)
        w2p[:, :PC, :] = w2[:, :, ps].transpose(0, 2, 1)
        # [L, 4(hcb), 128, 6(kc) * 512]
        w2r = w2p.reshape(L, KP, 128, 4, 512).transpose(0, 3, 2, 1, 4) \
                 .reshape(L, 4, 128, KP * 512)
        m["w2T"] = np.ascontiguousarray(w2r).astype(f16)
        owT = ow[vs, :].T  # [H, 4000]
        m["owT"] = np.stack(
            [pmajor(np.ascontiguousarray(owT[:, n * 250:(n + 1) * 250]))
             for n in range(16)]).astype(f16)
        in_maps.append(m)
    return in_maps


def kernel(**inputs) -> np.ndarray:
    from concourse import bass_utils

    if "nc" not in _STATE:
        _STATE["nc"] = _build()
    in_maps = _shard(inputs)
    res = bass_utils.run_bass_kernel_spmd(
        _STATE["nc"], in_maps, core_ids=list(range(NC)))
    out = np.concatenate(
        [res.results[c]["logits"] for c in range(NC)], axis=1)
    return out.astype(np.float32)


# revision 23
# speedup vs baseline: 1.0727x; 1.0727x over previous
"""Trainium2 Bass kernel: 4-layer decoder prefill (S=1024, H=2048, NH=16, HD=128,
FFN=5632, V=32000), tensor-parallel over 8 NeuronCores.

v2 (rewrite of the f32r baseline):
- All weights stream as float16 (half the HBM traffic); activations q/k/v/attn/
  swig/probabilities are float16; the residual stream, norms and PSUM stay f32.
- Causal skip: upper-triangle key blocks are never computed; the 4 distinct
  diagonal triangle masks live in SBUF (no per-layer mask DMA).
- Weights load via a few large 3D-AP DMAs (wqkv/wo resident per layer,
  w13/w2 streamed in big tiles); AR staging is coalesced to 1 DMA per 4
  h-chunks.  Weight DMAs issue on the SP ring, AR staging on the ACT ring.
- AllReduce replaces ReduceScatter+AllGather.
- Last layer computes only K/V for all tokens + everything else 2-wide for the
  final token; w2/w13 accumulate 2-wide outputs packed into single PSUM banks.
"""

import os
import sys

sys.path.insert(0, "/opt/trn_rl_repo")

import numpy as np

L = int(os.environ.get("KERNEL_DEV_L", "4"))
B, S, H, NH, HD = 1, 1024, 2048, 16, 128
V, P = 32000, 5632
NC = 8
FEAT = H // NC          # 256 q/k/v features per core (2 heads)
PC = P // NC            # 704 ffn rows per core
PCP = 768               # padded to 6*128
VC = V // NC            # 4000 vocab rows per core
KH = H // 128           # 16 H-chunks
KP = PCP // 128         # 6 ffn chunks
EPS = 1e-5
SCALE = float(np.sqrt(HD))
INV_SCALE = 1.0 / SCALE

_STATE = {}


def _build():
    import concourse.bass as bass
    import concourse.bacc as bacc
    from concourse import tile, mybir

    F32 = mybir.dt.float32
    F32R = mybir.dt.float32r
    F16 = mybir.dt.float16
    AF = mybir.ActivationFunctionType
    ALU = mybir.AluOpType
    ts = bass.ts

    nc = bacc.Bacc("TRN2", target_bir_lowering=False, debug=False, num_devices=NC)

    xT_h = nc.dram_tensor("xT", [2, 128, KH * 512], F32, kind="ExternalInput")
    tri_h = nc.dram_tensor("tri", [128, 4 * 512], F16, kind="ExternalInput")
    C_h = nc.dram_tensor("Cr", [128, S], F16, kind="ExternalInput")
    S_h = nc.dram_tensor("Sr", [128, S], F16, kind="ExternalInput")
    J_h = nc.dram_tensor("J", [128, 128], F16, kind="ExternalInput")
    id_h = nc.dram_tensor("ident", [128, 128], F16, kind="ExternalInput")
    n1w_h = nc.dram_tensor("n1w", [128, L * KH], F32, kind="ExternalInput")
    n2w_h = nc.dram_tensor("n2w", [128, L * KH], F32, kind="ExternalInput")
    fw_h = nc.dram_tensor("fw", [128, KH], F32, kind="ExternalInput")
    # all weights pre-swizzled on the host into their exact SBUF layouts so
    # every DMA is a contiguous full-partition-line read
    wqkv_h = nc.dram_tensor("wqkvT", [L, 128, KH * 3 * FEAT], F16,
                            kind="ExternalInput")
    woT_h = nc.dram_tensor("woT", [L, 128, 2 * H], F16, kind="ExternalInput")
    # [mg][hc-pair][p][2 x (384 g | 384 u)]
    w13_h = nc.dram_tensor("w13T", [L, 2, KH // 2, 128, 2 * 768], F16,
                           kind="ExternalInput")
    # [hcb][p][kc*512]
    w2T_h = nc.dram_tensor("w2T", [L, 4, 128, KP * 512], F16,
                           kind="ExternalInput")
    owT_h = nc.dram_tensor("owT", [16, 128, KH * 250], F16,
                           kind="ExternalInput")
    out_h = nc.dram_tensor("logits", [1, VC], F32, kind="ExternalOutput")

    from contextlib import ExitStack

    with tile.TileContext(nc) as tc, ExitStack() as _ctx:
        ec = _ctx.enter_context
        p_resid = ec(tc.tile_pool(name="resid", bufs=1))
        p_const = ec(tc.tile_pool(name="consts", bufs=1))
        p_wqkv = ec(tc.tile_pool(name="wqkv", bufs=1))
        p_wo = ec(tc.tile_pool(name="wo", bufs=1))
        p_w13 = ec(tc.tile_pool(name="w13", bufs=3))
        p_w2 = ec(tc.tile_pool(name="w2", bufs=2))
        p_ow = ec(tc.tile_pool(name="ow", bufs=2))
        p_qkv = ec(tc.tile_pool(name="qkv", bufs=1))
        p_vs = ec(tc.tile_pool(name="vs", bufs=1))
        p_attn = ec(tc.tile_pool(name="attn", bufs=1))
        p_swig = ec(tc.tile_pool(name="swig", bufs=6))
        p_stage = ec(tc.tile_pool(name="stage", bufs=2))
        p_rd = ec(tc.tile_pool(name="rd", bufs=2))
        p_ns = ec(tc.tile_pool(name="ns", bufs=2))
        p_pt = ec(tc.tile_pool(name="pt", bufs=3))
        p_gs = ec(tc.tile_pool(name="gs", bufs=2))
        p_row = ec(tc.tile_pool(name="row", bufs=2))
        p_row1 = ec(tc.tile_pool(name="row1", bufs=1))
        p_t = ec(tc.tile_pool(name="tsc", bufs=2))
        psum = ec(tc.tile_pool(name="psum", bufs=8, space="PSUM"))
        dram = ec(tc.tile_pool(name="dram", bufs=4, space="DRAM"))

        def r3(ap, p=128):
            """[N*p, c] dram/sbuf AP -> [p, N, c] with partition first."""
            return ap.rearrange("(n p) c -> p n c", p=p)

        def sb3(ap, c):
            return ap.rearrange("p (n c) -> p n c", c=c)

        # ---------------- constants + residual ----------------
        xT = p_resid.tile([128, KH * S], F32, tag="xT")
        xT3 = xT[:].rearrange("p (hc s) -> p hc s", s=S)
        nc.sync.dma_start(
            xT3[:, :, ts(0, 512)],
            xT_h.ap()[0].rearrange("p (hc c) -> p hc c", c=512))

        # n1w first: it gates every layer-0 xn op (tiny DMA, huge latency win)
        n1w = p_const.tile([128, L * KH], F32, tag="n1w")
        nc.sync.dma_start(n1w[:], n1w_h.ap())
        wq_s0 = p_wqkv.tile([128, KH * 3 * FEAT], F16, tag="wqkv", name="wq0")
        nc.sync.dma_start(wq_s0[:], wqkv_h.ap()[0])
        wo_s0 = p_wo.tile([128, 2 * H], F16, tag="wo", name="wo0")
        nc.sync.dma_start(wo_s0[:], woT_h.ap()[0])

        # 2MB warm-up AllReduce: absorbs the cold ncfw + RDH-path cost while
        # the prologue computes (payload is garbage weights, output unused)
        warm_in = dram.tile([128, 8000], F16, tag="arin", name="wrm_in")
        warm_out = dram.tile([128, 8000], F16, tag="arout",
                             addr_space="Shared", name="wrm_out")
        for i in range(2):
            nc.sync.dma_start(warm_in[:, ts(i, 4000)], owT_h.ap()[i])
        nc.gpsimd.collective_compute(
            "AllReduce", ALU.add, replica_groups=[list(range(NC))],
            ins=[warm_in[:].opt()], outs=[warm_out[:].opt()])

        nc.sync.dma_start(
            xT3[:, :, ts(1, 512)],
            xT_h.ap()[1].rearrange("p (hc c) -> p hc c", c=512))
        tri_s = p_const.tile([128, 4 * 512], F16, tag="tri")
        nc.sync.dma_start(tri_s[:], tri_h.ap())
        C_s = p_const.tile([128, S], F16, tag="C")
        nc.sync.dma_start(C_s[:], C_h.ap())
        S_s = p_const.tile([128, S], F16, tag="S")
        nc.sync.dma_start(S_s[:], S_h.ap())
        J_r = p_const.tile([128, 128], F16, tag="J")
        nc.sync.dma_start(J_r[:], J_h.ap())
        id_r = p_const.tile([128, 128], F16, tag="id")
        nc.sync.dma_start(id_r[:], id_h.ap())
        n2w = p_const.tile([128, L * KH], F32, tag="n2w")
        nc.sync.dma_start(n2w[:], n2w_h.ap())
        fw_s = p_const.tile([128, KH], F32, tag="fw")
        nc.sync.dma_start(fw_s[:], fw_h.ap())

        ones_f = p_const.tile([128, 1], F32, tag="o1f")
        nc.vector.memset(ones_f[:], 1.0)
        ones_col = p_const.tile([128, 1], F16, tag="o1")
        nc.vector.tensor_copy(ones_col[:], ones_f[:])
        ones_rf = p_const.tile([1, 128], F32, tag="orf")
        nc.vector.memset(ones_rf[:], 1.0)
        ones_row = p_const.tile([1, 128], F32R, tag="or")
        nc.vector.tensor_copy(ones_row[:], ones_rf[:])
        onc_r = p_const.tile([128, 1], F32R, tag="o1r")
        nc.vector.tensor_copy(onc_r[:], ones_f[:])
        eps_t = p_const.tile([1, 1], F32, tag="eps")
        nc.vector.memset(eps_t[:], EPS)
        eps_p = p_const.tile([128, 1], F32, tag="epsp")
        nc.vector.memset(eps_p[:], EPS)
        ones_mf = p_const.tile([128, 128], F32, tag="omf")
        nc.vector.memset(ones_mf[:], 1.0)
        ones_mat = p_const.tile([128, 128], F16, tag="om")
        nc.vector.tensor_copy(ones_mat[:], ones_mf[:])

        # ---------------- per-layer weight loads ----------------
        def load_wqkv(l_):
            w = p_wqkv.tile([128, KH * 3 * FEAT], F16, tag="wqkv", name=f"wq{l_}")
            nc.sync.dma_start(w[:], wqkv_h.ap()[l_])
            return w

        def load_wo(l_):
            w = p_wo.tile([128, 2 * H], F16, tag="wo", name=f"wo{l_}")
            nc.sync.dma_start(w[:], woT_h.ap()[l_])
            return w

        def norm_half(w_tile, l_, tk, ar_out=None):
            """1/rms for tokens [tk*512, tk*512+512) broadcast into PSUM
            [128,512]; returns the psum tile (keep alive while using).
            If ar_out is given, the residual add for this half is fused in
            front of the square pass chunk by chunk."""
            ssum = psum.tile([1, 512], F32, tag="ps", name="ssum")
            for hc in range(KH):
                if ar_out is not None and hc % 2 == 0:
                    hcb = hc // 2
                    rd = p_rd.tile([128, 2 * 512], F16, tag="rd", name="rd")
                    nc.scalar.dma_start(rd[:], ar_out[:, ts(hcb, 1024)])
                    nc.vector.tensor_add(
                        xT3[:, hc:hc + 2, ts(tk, 512)],
                        xT3[:, hc:hc + 2, ts(tk, 512)],
                        rd[:].rearrange("p (j c) -> p j c", c=512))
                sq = p_pt.tile([128, 512], F16, tag="pt", name="sq")
                if hc % 2 == 0:
                    nc.vector.tensor_mul(sq[:], xT3[:, hc, ts(tk, 512)],
                                         xT3[:, hc, ts(tk, 512)])
                else:
                    nc.scalar.activation(sq[:], xT3[:, hc, ts(tk, 512)],
                                         AF.Square)
                nc.tensor.matmul(ssum[:], ones_col[:], sq[:],
                                 start=(hc == 0), stop=(hc == KH - 1))
            rms = p_row1.tile([1, 512], F32, tag="rms")
            nc.scalar.activation(rms[:], ssum[:], AF.Sqrt,
                                 bias=eps_t[:], scale=1.0 / H)
            inv = p_row1.tile([1, 512], F32, tag="inv")
            nc.vector.reciprocal_approx_fast(out=inv[:], in_=rms[:])
            invr = p_row1.tile([1, 512], F32R, tag="invr")
            with nc.allow_low_precision(reason="f32r cast of 1/rms"):
                nc.vector.tensor_copy(invr[:], inv[:])
            bc_ps = psum.tile([128, 512], F32, tag="ps", name="bcps")
            nc.tensor.matmul(bc_ps[:], ones_row[:], invr[:], start=True,
                             stop=True)
            return bc_ps

        def qkv_half(l_, tk, wq_s, q_s, k_s, vT_s, q_only_last=False,
                     ar_out=None):
            """QKV for token half tk of layer l_.  If q_only_last, compute q
            just for the final 2 tokens (last layer)."""
            bc = norm_half(n1w, l_, tk, ar_out=ar_out)
            kp = [psum.tile([128, 512], F32, tag="ps", name=f"kp{i}")
                  for i in range(2)]
            vp = [psum.tile([128, 512], F32, tag="ps", name=f"vp{i}")
                  for i in range(2)]
            if not q_only_last:
                qp = [psum.tile([128, 512], F32, tag="ps", name=f"qp{i}")
                      for i in range(2)]
            elif tk == 1:
                qp = [psum.tile([128, 2], F32, tag="ps", name=f"q2p{i}")
                      for i in range(2)]
            else:
                qp = None
            for hc in range(KH):
                xn = p_ns.tile([128, 512], F16, tag="ns", name="xn")
                nc.vector.scalar_tensor_tensor(
                    xn[:], xT3[:, hc, ts(tk, 512)],
                    n1w[:, l_ * KH + hc: l_ * KH + hc + 1],
                    bc[:], op0=ALU.mult, op1=ALU.mult)
                st, sp = (hc == 0), (hc == KH - 1)
                woff = hc * 3 * FEAT
                for mt in range(2):
                    nc.tensor.matmul(
                        kp[mt][:], wq_s[:, woff + 256 + mt * 128:
                                        woff + 384 + mt * 128],
                        xn[:], start=st, stop=sp)
                    nc.tensor.matmul(
                        vp[mt][:], wq_s[:, woff + 512 + mt * 128:
                                        woff + 640 + mt * 128],
                        xn[:], start=st, stop=sp)
                    if qp is not None and not q_only_last:
                        nc.tensor.matmul(
                            qp[mt][:], wq_s[:, woff + mt * 128: woff + (mt + 1) * 128],
                            xn[:], start=st, stop=sp)
                    elif qp is not None:
                        nc.tensor.matmul(
                            qp[mt][:],
                            wq_s[:, woff + mt * 128: woff + (mt + 1) * 128],
                            xn[:, 510:512], start=st, stop=sp)
            for mt in range(2):
                off = mt * S + tk * 512
                nc.scalar.activation(k_s[:, off:off + 512], kp[mt][:], AF.Copy)
                nc.scalar.activation(vT_s[:, off:off + 512], vp[mt][:], AF.Copy)
                if qp is not None and not q_only_last:
                    nc.scalar.activation(q_s[:, off:off + 512], qp[mt][:],
                                         AF.Copy)
                elif qp is not None:
                    nc.scalar.activation(q_s[:, mt * S + S - 2: mt * S + S],
                                         qp[mt][:], AF.Copy)

        def rope(t_s, cols=None):
            """In-place interleaved rope on [128, 2*S] f16 (cols: list of
            (col_off, width, cs_off) to process; default full)."""
            if cols is None:
                cols = [(mt * S + n * 512, 512, n * 512)
                        for mt in range(2) for n in range(2)]
            for off, w, cs in cols:
                j_ps = psum.tile([128, 512], F32, tag="ps", name="jps")
                nc.tensor.matmul(j_ps[:, :w], J_r[:], t_s[:, off:off + w],
                                 start=True, stop=True)
                tmp = p_t.tile([128, 512], F16, tag="rt", name="rtmp")
                nc.vector.tensor_mul(tmp[:, :w], C_s[:, cs:cs + w],
                                     t_s[:, off:off + w])
                nc.vector.tensor_mul(t_s[:, off:off + w], j_ps[:, :w],
                                     S_s[:, cs:cs + w])
                nc.vector.tensor_add(t_s[:, off:off + w], t_s[:, off:off + w],
                                     tmp[:, :w])

        def v_transpose(vT_s, v_s, tk):
            for mt in range(2):
                for tb in range(tk * 4, tk * 4 + 4):
                    tp = psum.tile([128, 128], F16, tag="ps", name="tp")
                    nc.tensor.transpose(
                        tp[:], vT_s[:, mt * S + tb * 128: mt * S + tb * 128 + 128],
                        id_r[:])
                    nc.scalar.activation(
                        v_s[:, tb * FEAT + mt * 128: tb * FEAT + mt * 128 + 128],
                        tp[:], AF.Copy)

        def rope_half(t_s, tk):
            rope(t_s, cols=[(mt * S + tk * 512, 512, tk * 512)
                            for mt in range(2)])

        def qkv_tail(l_, tk, wq, q_s, k_s, vT_s, v_s, ar_out=None):
            qkv_half(l_, tk, wq, q_s, k_s, vT_s,
                     q_only_last=(l_ == L - 1), ar_out=ar_out)
            if l_ == L - 1:
                if tk == 1:
                    rope(q_s, cols=[(mt * S + S - 2, 2, S - 2)
                                    for mt in range(2)])
            else:
                rope_half(q_s, tk)
            rope_half(k_s, tk)
            v_transpose(vT_s, v_s, tk)

        def attention(tk, h, q_s, k_s, v_s, attn_s):
            at_ps = psum.tile([128, 512], F32, tag="ps", name="atp")
            rs_ps = psum.tile([1, 512], F32, tag="ps", name="rsp")
            nkc = 4 * (tk + 1)
            for kc in range(nkc):
                sc_ps = psum.tile([128, 512], F32, tag="ps", name="scp")
                nc.tensor.matmul(
                    sc_ps[:], k_s[:, h * S + kc * 128: h * S + kc * 128 + 128],
                    q_s[:, h * S + tk * 512: h * S + tk * 512 + 512],
                    start=True, stop=True)
                pt = p_pt.tile([128, 512], F16, tag="pt", name="ptl")
                d = kc - 4 * tk
                if d >= 0:
                    nc.vector.scalar_tensor_tensor(
                        sc_ps[:], sc_ps[:], INV_SCALE, tri_s[:, ts(d, 512)],
                        op0=ALU.mult, op1=ALU.add)
                    nc.scalar.activation(pt[:], sc_ps[:], AF.Exp)
                else:
                    nc.scalar.activation(pt[:], sc_ps[:], AF.Exp,
                                         scale=INV_SCALE)
                st, sp = (kc == 0), (kc == nkc - 1)
                nc.tensor.matmul(
                    at_ps[:], v_s[:, kc * FEAT + h * 128: kc * FEAT + h * 128 + 128],
                    pt[:], start=st, stop=sp)
                nc.tensor.matmul(rs_ps[:], ones_col[:], pt[:], start=st, stop=sp)
            inv = p_row1.tile([1, 512], F32, tag="inv")
            rs_sb = p_row1.tile([1, 512], F32, tag="rms")
            nc.scalar.activation(rs_sb[:], rs_ps[:], AF.Copy)
            nc.vector.reciprocal_approx_fast(out=inv[:], in_=rs_sb[:])
            invr = p_row1.tile([1, 512], F32R, tag="invr")
            with nc.allow_low_precision(reason="f32r cast of 1/sum"):
                nc.vector.tensor_copy(invr[:], inv[:])
            ib_ps = psum.tile([128, 512], F32, tag="ps", name="ibp")
            nc.tensor.matmul(ib_ps[:], ones_row[:], invr[:], start=True,
                             stop=True)
            ib_s = p_gs.tile([128, 512], F16, tag="ib", name="ibs")
            nc.scalar.activation(ib_s[:], ib_ps[:], AF.Copy)
            nc.vector.tensor_mul(
                attn_s[:, h * S + tk * 512: h * S + tk * 512 + 512],
                at_ps[:], ib_s[:])

        def wo_stage(l_, tk, wo_s, attn_s):
            """wo projection for half tk -> staged f16 AR input in DRAM.
            AR buffers are p-major [128, KH*512] (layout-consistent across
            cores, so the elementwise AllReduce is still correct)."""
            ar_in = dram.tile([128, KH * 512], F16, tag="arin", name="arin")
            ar_out = dram.tile([128, KH * 512], F16, tag="arout",
                               addr_space="Shared", name="arout")
            for hcb in range(8):
                st_t = p_stage.tile([128, 2 * 512], F16, tag="st", name="st")
                for hh in range(2):
                    hc = hcb * 2 + hh
                    po = psum.tile([128, 512], F32, tag="ps", name="po")
                    for fc in range(2):
                        nc.tensor.matmul(
                            po[:], wo_s[:, fc * H + hc * 128: fc * H + hc * 128 + 128],
                            attn_s[:, fc * S + tk * 512: fc * S + tk * 512 + 512],
                            start=(fc == 0), stop=(fc == 1))
                    nc.scalar.activation(st_t[:, ts(hh, 512)], po[:], AF.Copy)
                nc.scalar.dma_start(ar_in[:, ts(hcb, 1024)], st_t[:])
            nc.gpsimd.collective_compute(
                "AllReduce", ALU.add, replica_groups=[list(range(NC))],
                ins=[ar_in[:].opt()], outs=[ar_out[:].opt()])
            return ar_out

        def resid_add(tk, ar_out):
            for hcb in range(8):
                rd = p_rd.tile([128, 2 * 512], F16, tag="rd", name="rd")
                nc.scalar.dma_start(rd[:], ar_out[:, ts(hcb, 1024)])
                nc.vector.tensor_add(
                    xT3[:, hcb * 2:hcb * 2 + 2, ts(tk, 512)],
                    xT3[:, hcb * 2:hcb * 2 + 2, ts(tk, 512)],
                    rd[:].rearrange("p (j c) -> p j c", c=512))

        def ffn_half(l_, tk, swig, ar_out=None):
            """norm2 + w1/w3 + swiglu for half tk (into swig[kc] f16 tiles)."""
            bc2 = norm_half(n2w, l_, tk, ar_out=ar_out)
            for mg in range(2):
                mts = [0, 1, 2] if mg == 0 else [3, 4, 5]
                gp = {mt: psum.tile([128, 512], F32, tag="ps", name=f"gp{mt}")
                      for mt in mts}
                up = {mt: psum.tile([128, 512], F32, tag="ps", name=f"up{mt}")
                      for mt in mts}
                for hp in range(KH // 2):
                    wt = p_w13.tile([128, 1536], F16, tag="w13", name="wt13")
                    nc.sync.dma_start(wt[:], w13_h.ap()[l_, mg, hp])
                    for sub in range(2):
                        hc = hp * 2 + sub
                        hn = p_ns.tile([128, 512], F16, tag="ns", name="hn")
                        nc.vector.scalar_tensor_tensor(
                            hn[:], xT3[:, hc, ts(tk, 512)],
                            n2w[:, l_ * KH + hc: l_ * KH + hc + 1],
                            bc2[:], op0=ALU.mult, op1=ALU.mult)
                        st, sp = (hc == 0), (hc == KH - 1)
                        for i, mt in enumerate(mts):
                            nc.tensor.matmul(
                                gp[mt][:], wt[:, sub * 768 + i * 128:
                                              sub * 768 + (i + 1) * 128],
                                hn[:], start=st, stop=sp)
                            nc.tensor.matmul(
                                up[mt][:], wt[:, sub * 768 + 384 + i * 128:
                                              sub * 768 + 384 + (i + 1) * 128],
                                hn[:], start=st, stop=sp)
                for mt in mts:
                    gs = p_gs.tile([128, 512], F16, tag="gs", name="gs")
                    nc.scalar.activation(gs[:], gp[mt][:], AF.Silu)
                    nc.vector.tensor_mul(swig[mt][:, ts(tk, 512)],
                                         up[mt][:], gs[:])

        def w2_stage(l_, tk, swig):
            ar_in = dram.tile([128, KH * 512], F16, tag="arin", name="ar2in")
            ar_out = dram.tile([128, KH * 512], F16, tag="arout",
                               addr_space="Shared", name="ar2out")
            for hcb in range(4):
                p2 = [psum.tile([128, 512], F32, tag="ps", name=f"p2{i}")
                      for i in range(4)]
                wt = p_w2.tile([128, KP * 512], F16, tag="w2", name="w2t")
                nc.sync.dma_start(wt[:], w2T_h.ap()[l_, hcb])
                for kc in range(KP):
                    for hh in range(4):
                        nc.tensor.matmul(
                            p2[hh][:], wt[:, kc * 512 + hh * 128:
                                          kc * 512 + (hh + 1) * 128],
                            swig[kc][:, ts(tk, 512)],
                            start=(kc == 0), stop=(kc == KP - 1))
                for sg in range(2):
                    st_t = p_stage.tile([128, 2 * 512], F16, tag="st",
                                        name="st2")
                    for hh in range(2):
                        nc.scalar.activation(st_t[:, ts(hh, 512)],
                                             p2[sg * 2 + hh][:], AF.Copy)
                    nc.scalar.dma_start(
                        ar_in[:, ts(hcb * 2 + sg, 1024)], st_t[:])
            nc.gpsimd.collective_compute(
                "AllReduce", ALU.add, replica_groups=[list(range(NC))],
                ins=[ar_in[:].opt()], outs=[ar_out[:].opt()])
            return ar_out

        # ================= prologue =================
        wq_s = wq_s0
        wo_s = wo_s0
        cur_q = p_qkv.tile([128, 2 * S], F16, tag="q", name="q0")
        cur_k = p_qkv.tile([128, 2 * S], F16, tag="k", name="k0")
        cur_vT = p_qkv.tile([128, 2 * S], F16, tag="vT", name="vT0")
        cur_v = p_vs.tile([128, 8 * FEAT], F16, tag="v", name="vs0")
        qkv_tail(0, 0, wq_s, cur_q, cur_k, cur_vT, cur_v)
        attn_s = p_attn.tile([128, 2 * S], F16, tag="attn", name="attn0")
        for h in range(2):
            attention(0, h, cur_q, cur_k, cur_v, attn_s)
        ar1_0 = wo_stage(0, 0, wo_s, attn_s)
        qkv_tail(0, 1, wq_s, cur_q, cur_k, cur_vT, cur_v)
        for h in range(2):
            attention(1, h, cur_q, cur_k, cur_v, attn_s)
        ar1_1 = wo_stage(0, 1, wo_s, attn_s)

        for l in range(L - 1):
            nxt_wq = load_wqkv(l + 1)
            nxt_wo = load_wo(l + 1)
            swig = [p_swig.tile([128, S], F16, tag="sw", name=f"swig{i}")
                    for i in range(KP)]
            ffn_half(l, 0, swig, ar_out=ar1_0)
            ar2_0 = w2_stage(l, 0, swig)
            ffn_half(l, 1, swig, ar_out=ar1_1)
            ar2_1 = w2_stage(l, 1, swig)

            nxt_q = p_qkv.tile([128, 2 * S], F16, tag="q", name="qn")
            nxt_k = p_qkv.tile([128, 2 * S], F16, tag="k", name="kn")
            nxt_vT = p_qkv.tile([128, 2 * S], F16, tag="vT", name="vTn")
            nxt_v = p_vs.tile([128, 8 * FEAT], F16, tag="v", name="vsn")
            qkv_tail(l + 1, 0, nxt_wq, nxt_q, nxt_k, nxt_vT, nxt_v,
                     ar_out=ar2_0)
            if l + 1 < L - 1:
                attn_s = p_attn.tile([128, 2 * S], F16, tag="attn",
                                     name="attn")
                for h in range(2):
                    attention(0, h, nxt_q, nxt_k, nxt_v, attn_s)
                ar1_0 = wo_stage(l + 1, 0, nxt_wo, attn_s)
                qkv_tail(l + 1, 1, nxt_wq, nxt_q, nxt_k, nxt_vT, nxt_v,
                         ar_out=ar2_1)
                for h in range(2):
                    attention(1, h, nxt_q, nxt_k, nxt_v, attn_s)
                ar1_1 = wo_stage(l + 1, 1, nxt_wo, attn_s)
            else:
                qkv_tail(l + 1, 1, nxt_wq, nxt_q, nxt_k, nxt_vT, nxt_v,
                         ar_out=ar2_1)
            cur_q, cur_k, cur_vT, cur_v = nxt_q, nxt_k, nxt_vT, nxt_v
            wq_s, wo_s = nxt_wq, nxt_wo

        # ================= last layer =================
        l = L - 1
        q_s, k_s, vT_s, v_s = cur_q, cur_k, cur_vT, cur_v
        if True:
            if True:
                # ---- single-query attention (2-wide) ----
                attn_s = p_attn.tile([128, 2 * S], F16, tag="attn", name="attnL")
                for h in range(2):
                    at1 = psum.tile([128, 2], F32, tag="ps", name="at1")
                    rs1 = psum.tile([128, 2], F32, tag="ps", name="rs1")
                    for kc in range(8):
                        sc1 = psum.tile([128, 2], F32, tag="ps", name="sc1")
                        nc.tensor.matmul(
                            sc1[:],
                            k_s[:, h * S + kc * 128: h * S + kc * 128 + 128],
                            q_s[:, h * S + S - 2: h * S + S],
                            start=True, stop=True)
                        pt1 = p_row.tile([128, 2], F16, tag="sm", name="pt1")
                        if kc == 7:
                            nc.vector.scalar_tensor_tensor(
                                sc1[:], sc1[:], INV_SCALE,
                                tri_s[:, 3 * 512 + 510: 3 * 512 + 512],
                                op0=ALU.mult, op1=ALU.add)
                            nc.scalar.activation(pt1[:], sc1[:], AF.Exp)
                        else:
                            nc.scalar.activation(pt1[:], sc1[:], AF.Exp,
                                                 scale=INV_SCALE)
                        st, sp = (kc == 0), (kc == 7)
                        nc.tensor.matmul(
                            at1[:],
                            v_s[:, kc * FEAT + h * 128: kc * FEAT + h * 128 + 128],
                            pt1[:], start=st, stop=sp)
                        nc.tensor.matmul(rs1[:], ones_mat[:], pt1[:],
                                         start=st, stop=sp)
                    inva = p_row.tile([128, 2], F32, tag="sm", name="inva")
                    nc.vector.reciprocal(inva[:], rs1[:])
                    nc.vector.tensor_mul(
                        attn_s[:, h * S + S - 2: h * S + S], at1[:], inva[:])

                # wo (resident) -> [H,2] AllReduce -> residual add
                ar_in = dram.tile([128, KH * 2], F16, tag="arinL",
                                  name="arinL")
                ar_out = dram.tile([128, KH * 2], F16, tag="aroutL",
                                   addr_space="Shared", name="aroutL")
                stL = p_row.tile([128, KH * 2], F16, tag="stL", name="stL")
                for hc in range(KH):
                    poL = psum.tile([128, 2], F32, tag="ps", name="poL")
                    for fc in range(2):
                        nc.tensor.matmul(
                            poL[:], wo_s[:, fc * H + hc * 128: fc * H + hc * 128 + 128],
                            attn_s[:, fc * S + S - 2: fc * S + S],
                            start=(fc == 0), stop=(fc == 1))
                    nc.scalar.activation(stL[:, ts(hc, 2)], poL[:], AF.Copy)
                nc.scalar.dma_start(ar_in[:], stL[:])
                nc.gpsimd.collective_compute(
                    "AllReduce", ALU.add, replica_groups=[list(range(NC))],
                    ins=[ar_in[:].opt()], outs=[ar_out[:].opt()])
                rdL = p_row.tile([128, KH * 2], F16, tag="stL", name="rdL")
                nc.scalar.dma_start(rdL[:], ar_out[:])
                nc.vector.tensor_add(
                    xT3[:, :, S - 2:S], xT3[:, :, S - 2:S],
                    rdL[:].rearrange("p (j c) -> p j c", c=2))

                # ---- norm2 + FFN on the last 2 tokens ----
                sqL = p_row.tile([128, 2 * KH], F16, tag="sql2")
                for hc in range(KH):
                    nc.vector.tensor_mul(sqL[:, 2 * hc:2 * hc + 2],
                                         xT3[:, hc, S - 2:S],
                                         xT3[:, hc, S - 2:S])
                ssL = psum.tile([128, 2 * KH], F32, tag="ps", name="ssL")
                nc.tensor.matmul(ssL[:], ones_mat[:], sqL[:],
                                 start=True, stop=True)
                ssr = p_row.tile([128, 2], F32, tag="ssr")
                nc.vector.reduce_sum(
                    ssr[:], ssL[:].rearrange("p (c two) -> p two c", two=2),
                    axis=mybir.AxisListType.X)
                rmsL = p_row.tile([128, 2], F32, tag="rmsL")
                nc.scalar.activation(rmsL[:], ssr[:], AF.Sqrt,
                                     bias=eps_p[:], scale=1.0 / H)
                invL = p_row.tile([128, 2], F32, tag="invLc")
                nc.vector.reciprocal(invL[:], rmsL[:])
                hnL = p_row.tile([128, 2 * KH], F16, tag="hnL")
                tnL = p_row.tile([128, 2], F32, tag="tnL")
                for hc in range(KH):
                    nc.vector.tensor_scalar_mul(
                        tnL[:], xT3[:, hc, S - 2:S],
                        n2w[:, l * KH + hc: l * KH + hc + 1])
                    nc.vector.tensor_mul(hnL[:, 2 * hc:2 * hc + 2],
                                         tnL[:], invL[:])
                # w1/w3 flipped: stationary 2-wide hnL, moving w13 cols
                gT = [psum.tile([2, 384], F32, tag="ps", name=f"gT{i}")
                      for i in range(2)]
                uT = [psum.tile([2, 384], F32, tag="ps", name=f"uT{i}")
                      for i in range(2)]
                for mg in range(2):
                    for hp in range(KH // 2):
                        wt = p_w13.tile([128, 1536], F16, tag="w13",
                                        name="wt13L")
                        nc.sync.dma_start(wt[:], w13_h.ap()[l, mg, hp])
                        for sub in range(2):
                            hc = hp * 2 + sub
                            st_, sp_ = (hc == 0), (hc == KH - 1)
                            nc.tensor.matmul(
                                gT[mg][:], hnL[:, 2 * hc:2 * hc + 2],
                                wt[:, sub * 768: sub * 768 + 384],
                                start=st_, stop=sp_)
                            nc.tensor.matmul(
                                uT[mg][:], hnL[:, 2 * hc:2 * hc + 2],
                                wt[:, sub * 768 + 384: sub * 768 + 768],
                                start=st_, stop=sp_)
                gsT = p_row1.tile([2, 768], F16, tag="gsT")
                swT = p_row1.tile([2, 768], F16, tag="swT")
                for mg in range(2):
                    nc.scalar.activation(gsT[:, ts(mg, 384)], gT[mg][:],
                                         AF.Silu)
                    nc.vector.tensor_mul(swT[:, ts(mg, 384)], uT[mg][:],
                                         gsT[:, ts(mg, 384)])
                # transpose swigT [2,768] -> swigL [128, 12] (kc-major 2-wide)
                swigL = p_row.tile([128, 12], F16, tag="swL")
                for kc in range(KP):
                    tpL = psum.tile([128, 2], F16, tag="ps", name="tpL")
                    nc.tensor.transpose(tpL[:], swT[:, ts(kc, 128)],
                                        id_r[:2, :2])
                    nc.scalar.activation(swigL[:, ts(kc, 2)], tpL[:], AF.Copy)
                # w2 flipped: stationary 2-wide swigL, moving w2 hcb tiles
                ar2_in = dram.tile([128, KH * 2], F16, tag="arinL",
                                   name="ar2inL")
                ar2_out = dram.tile([128, KH * 2], F16, tag="aroutL",
                                    addr_space="Shared", name="ar2outL")
                p2T = [psum.tile([2, 512], F32, tag="ps", name=f"p2T{i}")
                       for i in range(4)]
                for hcb in range(4):
                    wt = p_w2.tile([128, KP * 512], F16, tag="w2", name="w2tL")
                    nc.sync.dma_start(wt[:], w2T_h.ap()[l, hcb])
                    for kc in range(KP):
                        nc.tensor.matmul(
                            p2T[hcb][:], swigL[:, ts(kc, 2)],
                            wt[:, ts(kc, 512)],
                            start=(kc == 0), stop=(kc == KP - 1))
                st2L = p_row.tile([128, KH * 2], F16, tag="stL", name="st2L")
                for hcb in range(4):
                    p2c = p_gs.tile([2, 512], F16, tag="p2c", name="p2c")
                    nc.scalar.activation(p2c[:], p2T[hcb][:], AF.Copy)
                    for hh in range(4):
                        hc = hcb * 4 + hh
                        tp2 = psum.tile([128, 2], F16, tag="ps", name="tp2")
                        nc.tensor.transpose(tp2[:], p2c[:, ts(hh, 128)],
                                            id_r[:2, :2])
                        nc.scalar.activation(st2L[:, ts(hc, 2)], tp2[:],
                                             AF.Copy)
                nc.scalar.dma_start(ar2_in[:], st2L[:])
                nc.gpsimd.collective_compute(
                    "AllReduce", ALU.add, replica_groups=[list(range(NC))],
                    ins=[ar2_in[:].opt()], outs=[ar2_out[:].opt()])
                rd2L = p_row.tile([128, KH * 2], F16, tag="stL", name="rd2L")
                nc.scalar.dma_start(rd2L[:], ar2_out[:])
                nc.vector.tensor_add(
                    xT3[:, :, S - 2:S], xT3[:, :, S - 2:S],
                    rd2L[:].rearrange("p (j c) -> p j c", c=2))
        # ======== final norm (last token only) + logits ========
        sq_l = p_row.tile([128, KH], F32R, tag="sql")
        for hc in range(KH):
            nc.vector.tensor_mul(sq_l[:, hc:hc + 1], xT3[:, hc, S - 1:S],
                                 xT3[:, hc, S - 1:S])
        sl_ps = psum.tile([1, KH], F32, tag="ps", name="slps")
        nc.tensor.matmul(sl_ps[:], onc_r[:], sq_l[:], start=True, stop=True)
        ssc = p_row.tile([1, 1], F32, tag="ssc")
        nc.vector.reduce_sum(ssc[:], sl_ps[:], axis=mybir.AxisListType.X)
        rms_l = p_row.tile([1, 1], F32, tag="rmsl")
        nc.scalar.activation(rms_l[:], ssc[:], AF.Sqrt, bias=eps_t[:],
                             scale=1.0 / H)
        inv_l = p_row.tile([1, 1], F32, tag="invl")
        nc.vector.reciprocal(inv_l[:], rms_l[:])
        xnl = p_row.tile([128, KH], F16, tag="xnl")
        for hc in range(KH):
            nc.vector.tensor_mul(xnl[:, hc:hc + 1], xT3[:, hc, S - 1:S],
                                 fw_s[:, hc:hc + 1])
        NCHUNK = 16
        CW = VC // NCHUNK  # 250
        for n in range(NCHUNK):
            ow_t = p_ow.tile([128, KH * CW], F16, tag="ow", name="owt")
            nc.sync.dma_start(ow_t[:], owT_h.ap()[n])
            lg_ps = psum.tile([1, CW], F32, tag="ps", name="lgps")
            for hc in range(KH):
                nc.tensor.matmul(lg_ps[:], xnl[:, hc: hc + 1],
                                 ow_t[:, ts(hc, CW)],
                                 start=(hc == 0), stop=(hc == KH - 1))
            lg = p_row1.tile([1, CW], F32, tag="lg")
            nc.scalar.activation(lg[:], lg_ps[:], AF.Copy, scale=inv_l[:])
            nc.sync.dma_start(out_h.ap()[:, ts(n, CW)], lg[:])

    nc.compile()
    return nc


def _shard(inputs):
    f16 = np.float16
    x = np.asarray(inputs["x"], np.float32)
    mask = np.asarray(inputs["attn_mask"], np.float32)
    cos = np.asarray(inputs["cos"], np.float32).reshape(S, HD // 2)
    sin = np.asarray(inputs["sin"], np.float32).reshape(S, HD // 2)
    n1 = np.asarray(inputs["norm1_w"], np.float32)[:L]
    n2 = np.asarray(inputs["norm2_w"], np.float32)[:L]
    fw = np.asarray(inputs["final_norm_w"], np.float32)
    wq = np.asarray(inputs["wq"], np.float32)[:L]
    wk = np.asarray(inputs["wk"], np.float32)[:L]
    wv = np.asarray(inputs["wv"], np.float32)[:L]
    wo = np.asarray(inputs["wo"], np.float32)[:L]
    w1 = np.asarray(inputs["w1"], np.float32)[:L]
    w3 = np.asarray(inputs["w3"], np.float32)[:L]
    w2 = np.asarray(inputs["w2"], np.float32)[:L]
    ow = np.asarray(inputs["out_w"], np.float32)

    def pmajor(a):
        """[N*128, c] -> [128, N*c] (SBUF layout: partition-major)."""
        n = a.shape[0] // 128
        return np.ascontiguousarray(
            a.reshape(n, 128, a.shape[1]).transpose(1, 0, 2)
            .reshape(128, n * a.shape[1]))

    xTf = x[0].T  # [H, S]
    xT = np.stack([pmajor(np.ascontiguousarray(xTf[:, tk * 512:(tk + 1) * 512]))
                   for tk in range(2)])

    maskT = mask[0].T  # [key, query]
    tri = np.ascontiguousarray(
        np.concatenate([maskT[d * 128:(d + 1) * 128, 0:512] for d in range(4)],
                       axis=1))
    tri = np.clip(tri, -30000.0, 0.0).astype(f16)
    C = np.empty((128, S), np.float32)
    C[0::2] = cos.T
    C[1::2] = cos.T
    Sm = np.empty((128, S), np.float32)
    Sm[0::2] = -sin.T
    Sm[1::2] = sin.T
    C = C.astype(f16)
    Sm = Sm.astype(f16)
    J = np.zeros((128, 128), f16)
    idx = np.arange(0, 128, 2)
    J[idx, idx + 1] = 1.0
    J[idx + 1, idx] = 1.0
    ident = np.eye(128, dtype=f16)
    n1w = np.ascontiguousarray(
        n1.reshape(L, KH, 128).transpose(2, 0, 1).reshape(128, L * KH))
    n2w = np.ascontiguousarray(
        n2.reshape(L, KH, 128).transpose(2, 0, 1).reshape(128, L * KH))
    fwh = np.ascontiguousarray(fw.reshape(KH, 128).T)

    common = dict(xT=xT, tri=tri, Cr=C, Sr=Sm, J=J, ident=ident,
                  n1w=n1w, n2w=n2w, fw=fwh)
    in_maps = []
    for c in range(NC):
        fs = slice(c * FEAT, (c + 1) * FEAT)
        ps = slice(c * PC, (c + 1) * PC)
        vs = slice(c * VC, (c + 1) * VC)
        m = dict(common)
        wqT = wq[:, fs, :].transpose(0, 2, 1)
        wkT = wk[:, fs, :].transpose(0, 2, 1)
        wvT = wv[:, fs, :].transpose(0, 2, 1)
        wqkv = np.concatenate([wqT, wkT, wvT], axis=2)  # [L, H, 768]
        m["wqkvT"] = np.stack([pmajor(wqkv[l]) for l in range(L)]).astype(f16)
        woT = wo[:, :, fs].transpose(0, 2, 1)  # [L, FEAT, H]
        m["woT"] = np.stack([pmajor(woT[l]) for l in range(L)]).astype(f16)
        w1T = np.zeros((L, H, PCP), np.float32)
        w3T = np.zeros((L, H, PCP), np.float32)
        w1T[:, :, :PC] = w1[:, ps, :].transpose(0, 2, 1)
        w3T[:, :, :PC] = w3[:, ps, :].transpose(0, 2, 1)
        # [L, 2(mg), 8(hc-pair), 128, 2 x (384 g | 384 u)]
        w13 = np.empty((L, 2, KH // 2, 128, 1536), np.float32)
        for mg in range(2):
            g = w1T[:, :, mg * 384:(mg + 1) * 384]
            u = w3T[:, :, mg * 384:(mg + 1) * 384]
            gu = np.concatenate([g, u], axis=2)  # [L, H, 768]
            w13[:, mg] = gu.reshape(L, KH // 2, 2 * 128, 768) \
                           .reshape(L, KH // 2, 2, 128, 768) \
                           .transpose(0, 1, 3, 2, 4).reshape(L, KH // 2, 128, 1536)
        m["w13T"] = w13.astype(f16)
        w2p = np.zeros((L, PCP, H), np.float32)
        w2p[:, :PC, :] = w2[:, :, ps].transpose(0, 2, 1)
        # [L, 4(hcb), 128, 6(kc) * 512]
        w2r = w2p.reshape(L, KP, 128, 4, 512).transpose(0, 3, 2, 1, 4) \
                 .reshape(L, 4, 128, KP * 512)
        m["w2T"] = np.ascontiguousarray(w2r).astype(f16)
        owT = ow[vs, :].T  # [H, 4000]
        m["owT"] = np.stack(
            [pmajor(np.ascontiguousarray(owT[:, n * 250:(n + 1) * 250]))
             for n in range(16)]).astype(f16)
        in_maps.append(m)
    return in_maps


def kernel(**inputs) -> np.ndarray:
    from concourse import bass_utils

    if "nc" not in _STATE:
        _STATE["nc"] = _build()
    in_maps = _shard(inputs)
    res = bass_utils.run_bass_kernel_spmd(
        _STATE["nc"], in_maps, core_ids=list(range(NC)))
    out = np.concatenate(
        [res.results[c]["logits"] for c in range(NC)], axis=1)
    return out.astype(np.float32)


# revision 24
# speedup vs baseline: 1.0808x; 1.0075x over previous
"""Trainium2 Bass kernel: 4-layer decoder prefill (S=1024, H=2048, NH=16, HD=128,
FFN=5632, V=32000), tensor-parallel over 8 NeuronCores.

v2 (rewrite of the f32r baseline):
- All weights stream as float16 (half the HBM traffic); activations q/k/v/attn/
  swig/probabilities are float16; the residual stream, norms and PSUM stay f32.
- Causal skip: upper-triangle key blocks are never computed; the 4 distinct
  diagonal triangle masks live in SBUF (no per-layer mask DMA).
- Weights load via a few large 3D-AP DMAs (wqkv/wo resident per layer,
  w13/w2 streamed in big tiles); AR staging is coalesced to 1 DMA per 4
  h-chunks.  Weight DMAs issue on the SP ring, AR staging on the ACT ring.
- AllReduce replaces ReduceScatter+AllGather.
- Last layer computes only K/V for all tokens + everything else 2-wide for the
  final token; w2/w13 accumulate 2-wide outputs packed into single PSUM banks.
"""

import os
import sys

sys.path.insert(0, "/opt/trn_rl_repo")

import numpy as np

L = int(os.environ.get("KERNEL_DEV_L", "4"))
B, S, H, NH, HD = 1, 1024, 2048, 16, 128
V, P = 32000, 5632
NC = 8
FEAT = H // NC          # 256 q/k/v features per core (2 heads)
PC = P // NC            # 704 ffn rows per core
PCP = 768               # padded to 6*128
VC = V // NC            # 4000 vocab rows per core
KH = H // 128           # 16 H-chunks
KP = PCP // 128         # 6 ffn chunks
EPS = 1e-5
SCALE = float(np.sqrt(HD))
INV_SCALE = 1.0 / SCALE

_STATE = {}


def _build():
    import concourse.bass as bass
    import concourse.bacc as bacc
    from concourse import tile, mybir

    F32 = mybir.dt.float32
    F32R = mybir.dt.float32r
    F16 = mybir.dt.float16
    AF = mybir.ActivationFunctionType
    ALU = mybir.AluOpType
    ts = bass.ts

    nc = bacc.Bacc("TRN2", target_bir_lowering=False, debug=False, num_devices=NC)

    xT_h = nc.dram_tensor("xT", [2, 128, KH * 512], F32, kind="ExternalInput")
    tri_h = nc.dram_tensor("tri", [128, 4 * 512], F16, kind="ExternalInput")
    C_h = nc.dram_tensor("Cr", [128, S], F16, kind="ExternalInput")
    S_h = nc.dram_tensor("Sr", [128, S], F16, kind="ExternalInput")
    J_h = nc.dram_tensor("J", [128, 128], F16, kind="ExternalInput")
    id_h = nc.dram_tensor("ident", [128, 128], F16, kind="ExternalInput")
    n1w_h = nc.dram_tensor("n1w", [128, L * KH], F32, kind="ExternalInput")
    n2w_h = nc.dram_tensor("n2w", [128, L * KH], F32, kind="ExternalInput")
    fw_h = nc.dram_tensor("fw", [128, KH], F32, kind="ExternalInput")
    # all weights pre-swizzled on the host into their exact SBUF layouts so
    # every DMA is a contiguous full-partition-line read
    wqkv_h = nc.dram_tensor("wqkvT", [L, 128, KH * 3 * FEAT], F16,
                            kind="ExternalInput")
    woT_h = nc.dram_tensor("woT", [L, 128, 2 * H], F16, kind="ExternalInput")
    # [mg][hc-pair][p][2 x (384 g | 384 u)]
    w13_h = nc.dram_tensor("w13T", [L, 2, KH // 2, 128, 2 * 768], F16,
                           kind="ExternalInput")
    # [hcb][p][kc*512]
    w2T_h = nc.dram_tensor("w2T", [L, 4, 128, KP * 512], F16,
                           kind="ExternalInput")
    owT_h = nc.dram_tensor("owT", [16, 128, KH * 250], F16,
                           kind="ExternalInput")
    out_h = nc.dram_tensor("logits", [1, VC], F32, kind="ExternalOutput")

    from contextlib import ExitStack

    with tile.TileContext(nc) as tc, ExitStack() as _ctx:
        ec = _ctx.enter_context
        p_resid = ec(tc.tile_pool(name="resid", bufs=1))
        p_const = ec(tc.tile_pool(name="consts", bufs=1))
        p_wqkv = ec(tc.tile_pool(name="wqkv", bufs=1))
        p_wo = ec(tc.tile_pool(name="wo", bufs=1))
        p_w13 = ec(tc.tile_pool(name="w13", bufs=3))
        p_w2 = ec(tc.tile_pool(name="w2", bufs=2))
        p_ow = ec(tc.tile_pool(name="ow", bufs=2))
        p_qkv = ec(tc.tile_pool(name="qkv", bufs=1))
        p_vs = ec(tc.tile_pool(name="vs", bufs=1))
        p_attn = ec(tc.tile_pool(name="attn", bufs=1))
        p_swig = ec(tc.tile_pool(name="swig", bufs=6))
        p_stage = ec(tc.tile_pool(name="stage", bufs=2))
        p_rd = ec(tc.tile_pool(name="rd", bufs=2))
        p_ns = ec(tc.tile_pool(name="ns", bufs=2))
        p_pt = ec(tc.tile_pool(name="pt", bufs=3))
        p_gs = ec(tc.tile_pool(name="gs", bufs=2))
        p_row = ec(tc.tile_pool(name="row", bufs=2))
        p_row1 = ec(tc.tile_pool(name="row1", bufs=1))
        p_t = ec(tc.tile_pool(name="tsc", bufs=2))
        psum = ec(tc.tile_pool(name="psum", bufs=8, space="PSUM"))
        dram = ec(tc.tile_pool(name="dram", bufs=4, space="DRAM"))

        def r3(ap, p=128):
            """[N*p, c] dram/sbuf AP -> [p, N, c] with partition first."""
            return ap.rearrange("(n p) c -> p n c", p=p)

        def sb3(ap, c):
            return ap.rearrange("p (n c) -> p n c", c=c)

        # ---------------- constants + residual ----------------
        xT = p_resid.tile([128, KH * S], F32, tag="xT")
        xT3 = xT[:].rearrange("p (hc s) -> p hc s", s=S)
        nc.sync.dma_start(
            xT3[:, :, ts(0, 512)],
            xT_h.ap()[0].rearrange("p (hc c) -> p hc c", c=512))

        # n1w first: it gates every layer-0 xn op (tiny DMA, huge latency win)
        n1w = p_const.tile([128, L * KH], F32, tag="n1w")
        nc.sync.dma_start(n1w[:], n1w_h.ap())
        wq_s0 = p_wqkv.tile([128, KH * 3 * FEAT], F16, tag="wqkv", name="wq0")
        nc.sync.dma_start(wq_s0[:], wqkv_h.ap()[0])
        wo_s0 = p_wo.tile([128, 2 * H], F16, tag="wo", name="wo0")
        nc.sync.dma_start(wo_s0[:], woT_h.ap()[0])

        # 2MB warm-up AllReduce: absorbs the cold ncfw + RDH-path cost while
        # the prologue computes (payload is garbage weights, output unused)
        warm_in = dram.tile([128, 8000], F16, tag="arin", name="wrm_in")
        warm_out = dram.tile([128, 8000], F16, tag="arout",
                             addr_space="Shared", name="wrm_out")
        for i in range(2):
            nc.sync.dma_start(warm_in[:, ts(i, 4000)], owT_h.ap()[i])
        nc.gpsimd.collective_compute(
            "AllReduce", ALU.add, replica_groups=[list(range(NC))],
            ins=[warm_in[:].opt()], outs=[warm_out[:].opt()])

        nc.sync.dma_start(
            xT3[:, :, ts(1, 512)],
            xT_h.ap()[1].rearrange("p (hc c) -> p hc c", c=512))
        tri_s = p_const.tile([128, 4 * 512], F16, tag="tri")
        nc.sync.dma_start(tri_s[:], tri_h.ap())
        C_s = p_const.tile([128, S], F16, tag="C")
        nc.sync.dma_start(C_s[:], C_h.ap())
        S_s = p_const.tile([128, S], F16, tag="S")
        nc.sync.dma_start(S_s[:], S_h.ap())
        J_r = p_const.tile([128, 128], F16, tag="J")
        nc.sync.dma_start(J_r[:], J_h.ap())
        id_r = p_const.tile([128, 128], F16, tag="id")
        nc.sync.dma_start(id_r[:], id_h.ap())
        n2w = p_const.tile([128, L * KH], F32, tag="n2w")
        nc.sync.dma_start(n2w[:], n2w_h.ap())
        fw_s = p_const.tile([128, KH], F32, tag="fw")
        nc.sync.dma_start(fw_s[:], fw_h.ap())

        ones_f = p_const.tile([128, 1], F32, tag="o1f")
        nc.vector.memset(ones_f[:], 1.0)
        ones_col = p_const.tile([128, 1], F16, tag="o1")
        nc.vector.tensor_copy(ones_col[:], ones_f[:])
        ones_rf = p_const.tile([1, 128], F32, tag="orf")
        nc.vector.memset(ones_rf[:], 1.0)
        ones_row = p_const.tile([1, 128], F32R, tag="or")
        nc.vector.tensor_copy(ones_row[:], ones_rf[:])
        onc_r = p_const.tile([128, 1], F32R, tag="o1r")
        nc.vector.tensor_copy(onc_r[:], ones_f[:])
        eps_t = p_const.tile([1, 1], F32, tag="eps")
        nc.vector.memset(eps_t[:], EPS)
        eps_p = p_const.tile([128, 1], F32, tag="epsp")
        nc.vector.memset(eps_p[:], EPS)
        ones_mf = p_const.tile([128, 128], F32, tag="omf")
        nc.vector.memset(ones_mf[:], 1.0)
        ones_mat = p_const.tile([128, 128], F16, tag="om")
        nc.vector.tensor_copy(ones_mat[:], ones_mf[:])

        # ---------------- per-layer weight loads ----------------
        def load_wqkv(l_):
            w = p_wqkv.tile([128, KH * 3 * FEAT], F16, tag="wqkv", name=f"wq{l_}")
            nc.sync.dma_start(w[:], wqkv_h.ap()[l_])
            return w

        def load_wo(l_):
            w = p_wo.tile([128, 2 * H], F16, tag="wo", name=f"wo{l_}")
            nc.sync.dma_start(w[:], woT_h.ap()[l_])
            return w

        def norm_half(w_tile, l_, tk, ar_out=None):
            """1/rms for tokens [tk*512, tk*512+512) broadcast into PSUM
            [128,512]; returns the psum tile (keep alive while using).
            If ar_out is given, the residual add for this half is fused in
            front of the square pass chunk by chunk."""
            ssum = psum.tile([1, 512], F32, tag="ps", name="ssum")
            for hc in range(KH):
                if ar_out is not None and hc % 2 == 0:
                    hcb = hc // 2
                    rd = p_rd.tile([128, 2 * 512], F16, tag="rd", name="rd")
                    nc.scalar.dma_start(rd[:], ar_out[:, ts(hcb, 1024)])
                    nc.vector.tensor_add(
                        xT3[:, hc:hc + 2, ts(tk, 512)],
                        xT3[:, hc:hc + 2, ts(tk, 512)],
                        rd[:].rearrange("p (j c) -> p j c", c=512))
                sq = p_pt.tile([128, 512], F16, tag="pt", name="sq")
                if hc % 2 == 0:
                    nc.vector.tensor_mul(sq[:], xT3[:, hc, ts(tk, 512)],
                                         xT3[:, hc, ts(tk, 512)])
                else:
                    nc.scalar.activation(sq[:], xT3[:, hc, ts(tk, 512)],
                                         AF.Square)
                nc.tensor.matmul(ssum[:], ones_col[:], sq[:],
                                 start=(hc == 0), stop=(hc == KH - 1))
            rms = p_row1.tile([1, 512], F32, tag="rms")
            nc.scalar.activation(rms[:], ssum[:], AF.Sqrt,
                                 bias=eps_t[:], scale=1.0 / H)
            inv = p_row1.tile([1, 512], F32, tag="inv")
            nc.vector.reciprocal_approx_fast(out=inv[:], in_=rms[:])
            invr = p_row1.tile([1, 512], F32R, tag="invr")
            with nc.allow_low_precision(reason="f32r cast of 1/rms"):
                nc.vector.tensor_copy(invr[:], inv[:])
            bc_ps = psum.tile([128, 512], F32, tag="ps", name="bcps")
            nc.tensor.matmul(bc_ps[:], ones_row[:], invr[:], start=True,
                             stop=True)
            return bc_ps

        def qkv_half(l_, tk, wq_s, q_s, k_s, vT_s, q_only_last=False,
                     ar_out=None):
            """QKV for token half tk of layer l_.  If q_only_last, compute q
            just for the final 2 tokens (last layer)."""
            bc = norm_half(n1w, l_, tk, ar_out=ar_out)
            kp = [psum.tile([128, 512], F32, tag="ps", name=f"kp{i}")
                  for i in range(2)]
            vp = [psum.tile([128, 512], F32, tag="ps", name=f"vp{i}")
                  for i in range(2)]
            if not q_only_last:
                qp = [psum.tile([128, 512], F32, tag="ps", name=f"qp{i}")
                      for i in range(2)]
            elif tk == 1:
                qp = [psum.tile([128, 2], F32, tag="ps", name=f"q2p{i}")
                      for i in range(2)]
            else:
                qp = None
            for hc in range(KH):
                xn = p_ns.tile([128, 512], F16, tag="ns", name="xn")
                nc.vector.scalar_tensor_tensor(
                    xn[:], xT3[:, hc, ts(tk, 512)],
                    n1w[:, l_ * KH + hc: l_ * KH + hc + 1],
                    bc[:], op0=ALU.mult, op1=ALU.mult)
                st, sp = (hc == 0), (hc == KH - 1)
                woff = hc * 3 * FEAT
                for mt in range(2):
                    nc.tensor.matmul(
                        kp[mt][:], wq_s[:, woff + 256 + mt * 128:
                                        woff + 384 + mt * 128],
                        xn[:], start=st, stop=sp)
                    nc.tensor.matmul(
                        vp[mt][:], wq_s[:, woff + 512 + mt * 128:
                                        woff + 640 + mt * 128],
                        xn[:], start=st, stop=sp)
                    if qp is not None and not q_only_last:
                        nc.tensor.matmul(
                            qp[mt][:], wq_s[:, woff + mt * 128: woff + (mt + 1) * 128],
                            xn[:], start=st, stop=sp)
                    elif qp is not None:
                        nc.tensor.matmul(
                            qp[mt][:],
                            wq_s[:, woff + mt * 128: woff + (mt + 1) * 128],
                            xn[:, 510:512], start=st, stop=sp)
            for mt in range(2):
                off = mt * S + tk * 512
                nc.scalar.activation(k_s[:, off:off + 512], kp[mt][:], AF.Copy)
                nc.scalar.activation(vT_s[:, off:off + 512], vp[mt][:], AF.Copy)
                if qp is not None and not q_only_last:
                    nc.scalar.activation(q_s[:, off:off + 512], qp[mt][:],
                                         AF.Copy)
                elif qp is not None:
                    nc.scalar.activation(q_s[:, mt * S + S - 2: mt * S + S],
                                         qp[mt][:], AF.Copy)

        def rope(t_s, cols=None):
            """In-place interleaved rope on [128, 2*S] f16 (cols: list of
            (col_off, width, cs_off) to process; default full)."""
            if cols is None:
                cols = [(mt * S + n * 512, 512, n * 512)
                        for mt in range(2) for n in range(2)]
            for off, w, cs in cols:
                j_ps = psum.tile([128, 512], F32, tag="ps", name="jps")
                nc.tensor.matmul(j_ps[:, :w], J_r[:], t_s[:, off:off + w],
                                 start=True, stop=True)
                tmp = p_t.tile([128, 512], F16, tag="rt", name="rtmp")
                nc.vector.tensor_mul(tmp[:, :w], C_s[:, cs:cs + w],
                                     t_s[:, off:off + w])
                nc.vector.tensor_mul(t_s[:, off:off + w], j_ps[:, :w],
                                     S_s[:, cs:cs + w])
                nc.vector.tensor_add(t_s[:, off:off + w], t_s[:, off:off + w],
                                     tmp[:, :w])

        def v_transpose(vT_s, v_s, tk):
            for mt in range(2):
                for tb in range(tk * 4, tk * 4 + 4):
                    tp = psum.tile([128, 128], F16, tag="ps", name="tp")
                    nc.tensor.transpose(
                        tp[:], vT_s[:, mt * S + tb * 128: mt * S + tb * 128 + 128],
                        id_r[:])
                    nc.scalar.activation(
                        v_s[:, tb * FEAT + mt * 128: tb * FEAT + mt * 128 + 128],
                        tp[:], AF.Copy)

        def rope_half(t_s, tk):
            rope(t_s, cols=[(mt * S + tk * 512, 512, tk * 512)
                            for mt in range(2)])

        def qkv_tail(l_, tk, wq, q_s, k_s, vT_s, v_s, ar_out=None):
            qkv_half(l_, tk, wq, q_s, k_s, vT_s,
                     q_only_last=(l_ == L - 1), ar_out=ar_out)
            if l_ == L - 1:
                if tk == 1:
                    rope(q_s, cols=[(mt * S + S - 2, 2, S - 2)
                                    for mt in range(2)])
            else:
                rope_half(q_s, tk)
            rope_half(k_s, tk)
            v_transpose(vT_s, v_s, tk)

        def attention(tk, h, q_s, k_s, v_s, attn_s):
            at_ps = psum.tile([128, 512], F32, tag="ps", name="atp")
            rs_ps = psum.tile([1, 512], F32, tag="ps", name="rsp")
            nkc = 4 * (tk + 1)
            for kc in range(nkc):
                sc_ps = psum.tile([128, 512], F32, tag="ps", name="scp")
                nc.tensor.matmul(
                    sc_ps[:], k_s[:, h * S + kc * 128: h * S + kc * 128 + 128],
                    q_s[:, h * S + tk * 512: h * S + tk * 512 + 512],
                    start=True, stop=True)
                pt = p_pt.tile([128, 512], F16, tag="pt", name="ptl")
                d = kc - 4 * tk
                if d >= 0:
                    nc.vector.scalar_tensor_tensor(
                        sc_ps[:], sc_ps[:], INV_SCALE, tri_s[:, ts(d, 512)],
                        op0=ALU.mult, op1=ALU.add)
                    nc.scalar.activation(pt[:], sc_ps[:], AF.Exp)
                else:
                    nc.scalar.activation(pt[:], sc_ps[:], AF.Exp,
                                         scale=INV_SCALE)
                st, sp = (kc == 0), (kc == nkc - 1)
                nc.tensor.matmul(
                    at_ps[:], v_s[:, kc * FEAT + h * 128: kc * FEAT + h * 128 + 128],
                    pt[:], start=st, stop=sp)
                nc.tensor.matmul(rs_ps[:], ones_col[:], pt[:], start=st, stop=sp)
            inv = p_row1.tile([1, 512], F32, tag="inv")
            rs_sb = p_row1.tile([1, 512], F32, tag="rms")
            nc.scalar.activation(rs_sb[:], rs_ps[:], AF.Copy)
            nc.vector.reciprocal_approx_fast(out=inv[:], in_=rs_sb[:])
            invr = p_row1.tile([1, 512], F32R, tag="invr")
            with nc.allow_low_precision(reason="f32r cast of 1/sum"):
                nc.vector.tensor_copy(invr[:], inv[:])
            ib_ps = psum.tile([128, 512], F32, tag="ps", name="ibp")
            nc.tensor.matmul(ib_ps[:], ones_row[:], invr[:], start=True,
                             stop=True)
            ib_s = p_gs.tile([128, 512], F16, tag="ib", name="ibs")
            nc.scalar.activation(ib_s[:], ib_ps[:], AF.Copy)
            nc.vector.tensor_mul(
                attn_s[:, h * S + tk * 512: h * S + tk * 512 + 512],
                at_ps[:], ib_s[:])

        def wo_stage(l_, tk, wo_s, attn_s):
            """wo projection for half tk -> staged f16 AR input in DRAM.
            AR buffers are p-major [128, KH*512] (layout-consistent across
            cores, so the elementwise AllReduce is still correct)."""
            ar_in = dram.tile([128, KH * 512], F16, tag="arin", name="arin")
            ar_out = dram.tile([128, KH * 512], F16, tag="arout",
                               addr_space="Shared", name="arout")
            for hcb in range(8):
                st_t = p_stage.tile([128, 2 * 512], F16, tag="st", name="st")
                for hh in range(2):
                    hc = hcb * 2 + hh
                    po = psum.tile([128, 512], F32, tag="ps", name="po")
                    for fc in range(2):
                        nc.tensor.matmul(
                            po[:], wo_s[:, fc * H + hc * 128: fc * H + hc * 128 + 128],
                            attn_s[:, fc * S + tk * 512: fc * S + tk * 512 + 512],
                            start=(fc == 0), stop=(fc == 1))
                    nc.scalar.activation(st_t[:, ts(hh, 512)], po[:], AF.Copy)
                nc.scalar.dma_start(ar_in[:, ts(hcb, 1024)], st_t[:])
            nc.gpsimd.collective_compute(
                "AllReduce", ALU.add, replica_groups=[list(range(NC))],
                ins=[ar_in[:].opt()], outs=[ar_out[:].opt()])
            return ar_out

        def resid_add(tk, ar_out):
            for hcb in range(8):
                rd = p_rd.tile([128, 2 * 512], F16, tag="rd", name="rd")
                nc.scalar.dma_start(rd[:], ar_out[:, ts(hcb, 1024)])
                nc.vector.tensor_add(
                    xT3[:, hcb * 2:hcb * 2 + 2, ts(tk, 512)],
                    xT3[:, hcb * 2:hcb * 2 + 2, ts(tk, 512)],
                    rd[:].rearrange("p (j c) -> p j c", c=512))

        def ffn_half(l_, tk, swig, ar_out=None):
            """norm2 + w1/w3 + swiglu for half tk (into swig[kc] f16 tiles)."""
            bc2 = norm_half(n2w, l_, tk, ar_out=ar_out)
            for mg in range(2):
                mts = [0, 1, 2] if mg == 0 else [3, 4, 5]
                gp = {mt: psum.tile([128, 512], F32, tag="ps", name=f"gp{mt}")
                      for mt in mts}
                up = {mt: psum.tile([128, 512], F32, tag="ps", name=f"up{mt}")
                      for mt in mts}
                for hp in range(KH // 2):
                    wt = p_w13.tile([128, 1536], F16, tag="w13", name="wt13")
                    nc.sync.dma_start(wt[:], w13_h.ap()[l_, mg, hp])
                    for sub in range(2):
                        hc = hp * 2 + sub
                        hn = p_ns.tile([128, 512], F16, tag="ns", name="hn")
                        nc.vector.scalar_tensor_tensor(
                            hn[:], xT3[:, hc, ts(tk, 512)],
                            n2w[:, l_ * KH + hc: l_ * KH + hc + 1],
                            bc2[:], op0=ALU.mult, op1=ALU.mult)
                        st, sp = (hc == 0), (hc == KH - 1)
                        for i, mt in enumerate(mts):
                            nc.tensor.matmul(
                                gp[mt][:], wt[:, sub * 768 + i * 128:
                                              sub * 768 + (i + 1) * 128],
                                hn[:], start=st, stop=sp)
                            nc.tensor.matmul(
                                up[mt][:], wt[:, sub * 768 + 384 + i * 128:
                                              sub * 768 + 384 + (i + 1) * 128],
                                hn[:], start=st, stop=sp)
                for mt in mts:
                    gs = p_gs.tile([128, 512], F16, tag="gs", name="gs")
                    nc.scalar.activation(gs[:], gp[mt][:], AF.Silu)
                    nc.vector.tensor_mul(swig[mt][:, ts(tk, 512)],
                                         up[mt][:], gs[:])

        def w2_stage(l_, tk, swig):
            ar_in = dram.tile([128, KH * 512], F16, tag="arin", name="ar2in")
            ar_out = dram.tile([128, KH * 512], F16, tag="arout",
                               addr_space="Shared", name="ar2out")
            for hcb in range(4):
                p2 = [psum.tile([128, 512], F32, tag="ps", name=f"p2{i}")
                      for i in range(4)]
                wt = p_w2.tile([128, KP * 512], F16, tag="w2", name="w2t")
                nc.sync.dma_start(wt[:], w2T_h.ap()[l_, hcb])
                for kc in range(KP):
                    for hh in range(4):
                        nc.tensor.matmul(
                            p2[hh][:], wt[:, kc * 512 + hh * 128:
                                          kc * 512 + (hh + 1) * 128],
                            swig[kc][:, ts(tk, 512)],
                            start=(kc == 0), stop=(kc == KP - 1))
                for sg in range(2):
                    st_t = p_stage.tile([128, 2 * 512], F16, tag="st",
                                        name="st2")
                    for hh in range(2):
                        nc.scalar.activation(st_t[:, ts(hh, 512)],
                                             p2[sg * 2 + hh][:], AF.Copy)
                    nc.scalar.dma_start(
                        ar_in[:, ts(hcb * 2 + sg, 1024)], st_t[:])
            nc.gpsimd.collective_compute(
                "AllReduce", ALU.add, replica_groups=[list(range(NC))],
                ins=[ar_in[:].opt()], outs=[ar_out[:].opt()])
            return ar_out

        # ================= prologue =================
        wq_s = wq_s0
        wo_s = wo_s0
        cur_q = p_qkv.tile([128, 2 * S], F16, tag="q", name="q0")
        cur_k = p_qkv.tile([128, 2 * S], F16, tag="k", name="k0")
        cur_vT = p_qkv.tile([128, 2 * S], F16, tag="vT", name="vT0")
        cur_v = p_vs.tile([128, 8 * FEAT], F16, tag="v", name="vs0")
        qkv_tail(0, 0, wq_s, cur_q, cur_k, cur_vT, cur_v)
        attn_s = p_attn.tile([128, 2 * S], F16, tag="attn", name="attn0")
        for h in range(2):
            attention(0, h, cur_q, cur_k, cur_v, attn_s)
        ar1_0 = wo_stage(0, 0, wo_s, attn_s)
        qkv_tail(0, 1, wq_s, cur_q, cur_k, cur_vT, cur_v)
        for h in range(2):
            attention(1, h, cur_q, cur_k, cur_v, attn_s)
        ar1_1 = wo_stage(0, 1, wo_s, attn_s)

        for l in range(L - 1):
            nxt_wq = load_wqkv(l + 1)
            nxt_wo = load_wo(l + 1)
            swig = [p_swig.tile([128, S], F16, tag="sw", name=f"swig{i}")
                    for i in range(KP)]
            ffn_half(l, 0, swig, ar_out=ar1_0)
            ar2_0 = w2_stage(l, 0, swig)
            ffn_half(l, 1, swig, ar_out=ar1_1)
            ar2_1 = w2_stage(l, 1, swig)

            nxt_q = p_qkv.tile([128, 2 * S], F16, tag="q", name="qn")
            nxt_k = p_qkv.tile([128, 2 * S], F16, tag="k", name="kn")
            nxt_vT = p_qkv.tile([128, 2 * S], F16, tag="vT", name="vTn")
            nxt_v = p_vs.tile([128, 8 * FEAT], F16, tag="v", name="vsn")
            qkv_tail(l + 1, 0, nxt_wq, nxt_q, nxt_k, nxt_vT, nxt_v,
                     ar_out=ar2_0)
            if l + 1 < L - 1:
                attn_s = p_attn.tile([128, 2 * S], F16, tag="attn",
                                     name="attn")
                for h in range(2):
                    attention(0, h, nxt_q, nxt_k, nxt_v, attn_s)
                ar1_0 = wo_stage(l + 1, 0, nxt_wo, attn_s)
                qkv_tail(l + 1, 1, nxt_wq, nxt_q, nxt_k, nxt_vT, nxt_v,
                         ar_out=ar2_1)
                for h in range(2):
                    attention(1, h, nxt_q, nxt_k, nxt_v, attn_s)
                ar1_1 = wo_stage(l + 1, 1, nxt_wo, attn_s)
            else:
                qkv_tail(l + 1, 1, nxt_wq, nxt_q, nxt_k, nxt_vT, nxt_v,
                         ar_out=ar2_1)
            cur_q, cur_k, cur_vT, cur_v = nxt_q, nxt_k, nxt_vT, nxt_v
            wq_s, wo_s = nxt_wq, nxt_wo

        _OW_PRE = []
        # ================= last layer =================
        l = L - 1
        q_s, k_s, vT_s, v_s = cur_q, cur_k, cur_vT, cur_v
        if True:
            if True:
                # ---- single-query attention (2-wide) ----
                attn_s = p_attn.tile([128, 2 * S], F16, tag="attn", name="attnL")
                for h in range(2):
                    at1 = psum.tile([128, 2], F32, tag="ps", name="at1")
                    rs1 = psum.tile([128, 2], F32, tag="ps", name="rs1")
                    for kc in range(8):
                        sc1 = psum.tile([128, 2], F32, tag="ps", name="sc1")
                        nc.tensor.matmul(
                            sc1[:],
                            k_s[:, h * S + kc * 128: h * S + kc * 128 + 128],
                            q_s[:, h * S + S - 2: h * S + S],
                            start=True, stop=True)
                        pt1 = p_row.tile([128, 2], F16, tag="sm", name="pt1")
                        if kc == 7:
                            nc.vector.scalar_tensor_tensor(
                                sc1[:], sc1[:], INV_SCALE,
                                tri_s[:, 3 * 512 + 510: 3 * 512 + 512],
                                op0=ALU.mult, op1=ALU.add)
                            nc.scalar.activation(pt1[:], sc1[:], AF.Exp)
                        else:
                            nc.scalar.activation(pt1[:], sc1[:], AF.Exp,
                                                 scale=INV_SCALE)
                        st, sp = (kc == 0), (kc == 7)
                        nc.tensor.matmul(
                            at1[:],
                            v_s[:, kc * FEAT + h * 128: kc * FEAT + h * 128 + 128],
                            pt1[:], start=st, stop=sp)
                        nc.tensor.matmul(rs1[:], ones_mat[:], pt1[:],
                                         start=st, stop=sp)
                    inva = p_row.tile([128, 2], F32, tag="sm", name="inva")
                    nc.vector.reciprocal(inva[:], rs1[:])
                    nc.vector.tensor_mul(
                        attn_s[:, h * S + S - 2: h * S + S], at1[:], inva[:])

                # wo (resident) -> [H,2] AllReduce -> residual add
                ar_in = dram.tile([128, KH * 2], F16, tag="arinL",
                                  name="arinL")
                ar_out = dram.tile([128, KH * 2], F16, tag="aroutL",
                                   addr_space="Shared", name="aroutL")
                stL = p_row.tile([128, KH * 2], F16, tag="stL", name="stL")
                for hc in range(KH):
                    poL = psum.tile([128, 2], F32, tag="ps", name="poL")
                    for fc in range(2):
                        nc.tensor.matmul(
                            poL[:], wo_s[:, fc * H + hc * 128: fc * H + hc * 128 + 128],
                            attn_s[:, fc * S + S - 2: fc * S + S],
                            start=(fc == 0), stop=(fc == 1))
                    nc.scalar.activation(stL[:, ts(hc, 2)], poL[:], AF.Copy)
                nc.scalar.dma_start(ar_in[:], stL[:])
                nc.gpsimd.collective_compute(
                    "AllReduce", ALU.add, replica_groups=[list(range(NC))],
                    ins=[ar_in[:].opt()], outs=[ar_out[:].opt()])
                rdL = p_row.tile([128, KH * 2], F16, tag="stL", name="rdL")
                nc.scalar.dma_start(rdL[:], ar_out[:])
                nc.vector.tensor_add(
                    xT3[:, :, S - 2:S], xT3[:, :, S - 2:S],
                    rdL[:].rearrange("p (j c) -> p j c", c=2))

                # ---- norm2 + FFN on the last 2 tokens ----
                sqL = p_row.tile([128, 2 * KH], F16, tag="sql2")
                for hc in range(KH):
                    nc.vector.tensor_mul(sqL[:, 2 * hc:2 * hc + 2],
                                         xT3[:, hc, S - 2:S],
                                         xT3[:, hc, S - 2:S])
                ssL = psum.tile([128, 2 * KH], F32, tag="ps", name="ssL")
                nc.tensor.matmul(ssL[:], ones_mat[:], sqL[:],
                                 start=True, stop=True)
                ssr = p_row.tile([128, 2], F32, tag="ssr")
                nc.vector.reduce_sum(
                    ssr[:], ssL[:].rearrange("p (c two) -> p two c", two=2),
                    axis=mybir.AxisListType.X)
                rmsL = p_row.tile([128, 2], F32, tag="rmsL")
                nc.scalar.activation(rmsL[:], ssr[:], AF.Sqrt,
                                     bias=eps_p[:], scale=1.0 / H)
                invL = p_row.tile([128, 2], F32, tag="invLc")
                nc.vector.reciprocal(invL[:], rmsL[:])
                hnL = p_row.tile([128, 2 * KH], F16, tag="hnL")
                tnL = p_row.tile([128, 2], F32, tag="tnL")
                for hc in range(KH):
                    nc.vector.tensor_scalar_mul(
                        tnL[:], xT3[:, hc, S - 2:S],
                        n2w[:, l * KH + hc: l * KH + hc + 1])
                    nc.vector.tensor_mul(hnL[:, 2 * hc:2 * hc + 2],
                                         tnL[:], invL[:])
                # prefetch the first two logits weight chunks now: the
                # SP ring is idle here and the tiles sit until the final norm
                for _n in range(2):
                    _owt = p_ow.tile([128, KH * (VC // 16)], F16, tag="ow",
                                     name="owt")
                    nc.sync.dma_start(_owt[:], owT_h.ap()[_n])
                    _OW_PRE.append(_owt)
                # w1/w3 flipped: stationary 2-wide hnL, moving w13 cols
                gT = [psum.tile([2, 384], F32, tag="ps", name=f"gT{i}")
                      for i in range(2)]
                uT = [psum.tile([2, 384], F32, tag="ps", name=f"uT{i}")
                      for i in range(2)]
                for mg in range(2):
                    for hp in range(KH // 2):
                        wt = p_w13.tile([128, 1536], F16, tag="w13",
                                        name="wt13L")
                        nc.sync.dma_start(wt[:], w13_h.ap()[l, mg, hp])
                        for sub in range(2):
                            hc = hp * 2 + sub
                            st_, sp_ = (hc == 0), (hc == KH - 1)
                            nc.tensor.matmul(
                                gT[mg][:], hnL[:, 2 * hc:2 * hc + 2],
                                wt[:, sub * 768: sub * 768 + 384],
                                start=st_, stop=sp_)
                            nc.tensor.matmul(
                                uT[mg][:], hnL[:, 2 * hc:2 * hc + 2],
                                wt[:, sub * 768 + 384: sub * 768 + 768],
                                start=st_, stop=sp_)
                gsT = p_row1.tile([2, 768], F16, tag="gsT")
                swT = p_row1.tile([2, 768], F16, tag="swT")
                for mg in range(2):
                    nc.scalar.activation(gsT[:, ts(mg, 384)], gT[mg][:],
                                         AF.Silu)
                    nc.vector.tensor_mul(swT[:, ts(mg, 384)], uT[mg][:],
                                         gsT[:, ts(mg, 384)])
                # transpose swigT [2,768] -> swigL [128, 12] (kc-major 2-wide)
                swigL = p_row.tile([128, 12], F16, tag="swL")
                for kc in range(KP):
                    tpL = psum.tile([128, 2], F16, tag="ps", name="tpL")
                    nc.tensor.transpose(tpL[:], swT[:, ts(kc, 128)],
                                        id_r[:2, :2])
                    nc.scalar.activation(swigL[:, ts(kc, 2)], tpL[:], AF.Copy)
                # w2 flipped: stationary 2-wide swigL, moving w2 hcb tiles
                ar2_in = dram.tile([128, KH * 2], F16, tag="arinL",
                                   name="ar2inL")
                ar2_out = dram.tile([128, KH * 2], F16, tag="aroutL",
                                    addr_space="Shared", name="ar2outL")
                p2T = [psum.tile([2, 512], F32, tag="ps", name=f"p2T{i}")
                       for i in range(4)]
                for hcb in range(4):
                    wt = p_w2.tile([128, KP * 512], F16, tag="w2", name="w2tL")
                    nc.sync.dma_start(wt[:], w2T_h.ap()[l, hcb])
                    for kc in range(KP):
                        nc.tensor.matmul(
                            p2T[hcb][:], swigL[:, ts(kc, 2)],
                            wt[:, ts(kc, 512)],
                            start=(kc == 0), stop=(kc == KP - 1))
                st2L = p_row.tile([128, KH * 2], F16, tag="stL", name="st2L")
                for hcb in range(4):
                    p2c = p_gs.tile([2, 512], F16, tag="p2c", name="p2c")
                    nc.scalar.activation(p2c[:], p2T[hcb][:], AF.Copy)
                    for hh in range(4):
                        hc = hcb * 4 + hh
                        tp2 = psum.tile([128, 2], F16, tag="ps", name="tp2")
                        nc.tensor.transpose(tp2[:], p2c[:, ts(hh, 128)],
                                            id_r[:2, :2])
                        nc.scalar.activation(st2L[:, ts(hc, 2)], tp2[:],
                                             AF.Copy)
                nc.scalar.dma_start(ar2_in[:], st2L[:])
                nc.gpsimd.collective_compute(
                    "AllReduce", ALU.add, replica_groups=[list(range(NC))],
                    ins=[ar2_in[:].opt()], outs=[ar2_out[:].opt()])
                rd2L = p_row.tile([128, KH * 2], F16, tag="stL", name="rd2L")
                nc.scalar.dma_start(rd2L[:], ar2_out[:])
                nc.vector.tensor_add(
                    xT3[:, :, S - 2:S], xT3[:, :, S - 2:S],
                    rd2L[:].rearrange("p (j c) -> p j c", c=2))
        # ======== final norm (last token only) + logits ========
        sq_l = p_row.tile([128, KH], F32R, tag="sql")
        for hc in range(KH):
            nc.vector.tensor_mul(sq_l[:, hc:hc + 1], xT3[:, hc, S - 1:S],
                                 xT3[:, hc, S - 1:S])
        sl_ps = psum.tile([1, KH], F32, tag="ps", name="slps")
        nc.tensor.matmul(sl_ps[:], onc_r[:], sq_l[:], start=True, stop=True)
        ssc = p_row.tile([1, 1], F32, tag="ssc")
        nc.vector.reduce_sum(ssc[:], sl_ps[:], axis=mybir.AxisListType.X)
        rms_l = p_row.tile([1, 1], F32, tag="rmsl")
        nc.scalar.activation(rms_l[:], ssc[:], AF.Sqrt, bias=eps_t[:],
                             scale=1.0 / H)
        inv_l = p_row.tile([1, 1], F32, tag="invl")
        nc.vector.reciprocal(inv_l[:], rms_l[:])
        xnl = p_row.tile([128, KH], F16, tag="xnl")
        for hc in range(KH):
            nc.vector.tensor_mul(xnl[:, hc:hc + 1], xT3[:, hc, S - 1:S],
                                 fw_s[:, hc:hc + 1])
        NCHUNK = 16
        CW = VC // NCHUNK  # 250
        ow_tiles = {0: _OW_PRE[0], 1: _OW_PRE[1]}
        for n in range(NCHUNK):
            ow_t = ow_tiles.pop(n)
            if n + 2 < NCHUNK:
                ow_tiles[n + 2] = p_ow.tile([128, KH * CW], F16, tag="ow",
                                            name="owt")
                nc.sync.dma_start(ow_tiles[n + 2][:], owT_h.ap()[n + 2])
            lg_ps = psum.tile([1, CW], F32, tag="ps", name="lgps")
            for hc in range(KH):
                nc.tensor.matmul(lg_ps[:], xnl[:, hc: hc + 1],
                                 ow_t[:, ts(hc, CW)],
                                 start=(hc == 0), stop=(hc == KH - 1))
            lg = p_row1.tile([1, CW], F32, tag="lg")
            nc.scalar.activation(lg[:], lg_ps[:], AF.Copy, scale=inv_l[:])
            nc.sync.dma_start(out_h.ap()[:, ts(n, CW)], lg[:])

    nc.compile()
    return nc


def _shard(inputs):
    f16 = np.float16
    x = np.asarray(inputs["x"], np.float32)
    mask = np.asarray(inputs["attn_mask"], np.float32)
    cos = np.asarray(inputs["cos"], np.float32).reshape(S, HD // 2)
    sin = np.asarray(inputs["sin"], np.float32).reshape(S, HD // 2)
    n1 = np.asarray(inputs["norm1_w"], np.float32)[:L]
    n2 = np.asarray(inputs["norm2_w"], np.float32)[:L]
    fw = np.asarray(inputs["final_norm_w"], np.float32)
    wq = np.asarray(inputs["wq"], np.float32)[:L]
    wk = np.asarray(inputs["wk"], np.float32)[:L]
    wv = np.asarray(inputs["wv"], np.float32)[:L]
    wo = np.asarray(inputs["wo"], np.float32)[:L]
    w1 = np.asarray(inputs["w1"], np.float32)[:L]
    w3 = np.asarray(inputs["w3"], np.float32)[:L]
    w2 = np.asarray(inputs["w2"], np.float32)[:L]
    ow = np.asarray(inputs["out_w"], np.float32)

    def pmajor(a):
        """[N*128, c] -> [128, N*c] (SBUF layout: partition-major)."""
        n = a.shape[0] // 128
        return np.ascontiguousarray(
            a.reshape(n, 128, a.shape[1]).transpose(1, 0, 2)
            .reshape(128, n * a.shape[1]))

    xTf = x[0].T  # [H, S]
    xT = np.stack([pmajor(np.ascontiguousarray(xTf[:, tk * 512:(tk + 1) * 512]))
                   for tk in range(2)])

    maskT = mask[0].T  # [key, query]
    tri = np.ascontiguousarray(
        np.concatenate([maskT[d * 128:(d + 1) * 128, 0:512] for d in range(4)],
                       axis=1))
    tri = np.clip(tri, -30000.0, 0.0).astype(f16)
    C = np.empty((128, S), np.float32)
    C[0::2] = cos.T
    C[1::2] = cos.T
    Sm = np.empty((128, S), np.float32)
    Sm[0::2] = -sin.T
    Sm[1::2] = sin.T
    C = C.astype(f16)
    Sm = Sm.astype(f16)
    J = np.zeros((128, 128), f16)
    idx = np.arange(0, 128, 2)
    J[idx, idx + 1] = 1.0
    J[idx + 1, idx] = 1.0
    ident = np.eye(128, dtype=f16)
    n1w = np.ascontiguousarray(
        n1.reshape(L, KH, 128).transpose(2, 0, 1).reshape(128, L * KH))
    n2w = np.ascontiguousarray(
        n2.reshape(L, KH, 128).transpose(2, 0, 1).reshape(128, L * KH))
    fwh = np.ascontiguousarray(fw.reshape(KH, 128).T)

    common = dict(xT=xT, tri=tri, Cr=C, Sr=Sm, J=J, ident=ident,
                  n1w=n1w, n2w=n2w, fw=fwh)
    in_maps = []
    for c in range(NC):
        fs = slice(c * FEAT, (c + 1) * FEAT)
        ps = slice(c * PC, (c + 1) * PC)
        vs = slice(c * VC, (c + 1) * VC)
        m = dict(common)
        wqT = wq[:, fs, :].transpose(0, 2, 1)
        wkT = wk[:, fs, :].transpose(0, 2, 1)
        wvT = wv[:, fs, :].transpose(0, 2, 1)
        wqkv = np.concatenate([wqT, wkT, wvT], axis=2)  # [L, H, 768]
        m["wqkvT"] = np.stack([pmajor(wqkv[l]) for l in range(L)]).astype(f16)
        woT = wo[:, :, fs].transpose(0, 2, 1)  # [L, FEAT, H]
        m["woT"] = np.stack([pmajor(woT[l]) for l in range(L)]).astype(f16)
        w1T = np.zeros((L, H, PCP), np.float32)
        w3T = np.zeros((L, H, PCP), np.float32)
        w1T[:, :, :PC] = w1[:, ps, :].transpose(0, 2, 1)
        w3T[:, :, :PC] = w3[:, ps, :].transpose(0, 2, 1)
        # [L, 2(mg), 8(hc-pair), 128, 2 x (384 g | 384 u)]
        w13 = np.empty((L, 2, KH // 2, 128, 1536), np.float32)
        for mg in range(2):
            g = w1T[:, :, mg * 384:(mg + 1) * 384]
            u = w3T[:, :, mg * 384:(mg + 1) * 384]
            gu = np.concatenate([g, u], axis=2)  # [L, H, 768]
            w13[:, mg] = gu.reshape(L, KH // 2, 2 * 128, 768) \
                           .reshape(L, KH // 2, 2, 128, 768) \
                           .transpose(0, 1, 3, 2, 4).reshape(L, KH // 2, 128, 1536)
        m["w13T"] = w13.astype(f16)
        w2p = np.zeros((L, PCP, H), np.float32)
        w2p[:, :PC, :] = w2[:, :, ps].transpose(0, 2, 1)
        # [L, 4(hcb), 128, 6(kc) * 512]
        w2r = w2p.reshape(L, KP, 128, 4, 512).transpose(0, 3, 2, 1, 4) \
                 .reshape(L, 4, 128, KP * 512)
        m["w2T"] = np.ascontiguousarray(w2r).astype(f16)
        owT = ow[vs, :].T  # [H, 4000]
        m["owT"] = np.stack(
            [pmajor(np.ascontiguousarray(owT[:, n * 250:(n + 1) * 250]))
             for n in range(16)]).astype(f16)
        in_maps.append(m)
    return in_maps


def kernel(**inputs) -> np.ndarray:
    from concourse import bass_utils

    if "nc" not in _STATE:
        _STATE["nc"] = _build()
    in_maps = _shard(inputs)
    res = bass_utils.run_bass_kernel_spmd(
        _STATE["nc"], in_maps, core_ids=list(range(NC)))
    out = np.concatenate(
        [res.results[c]["logits"] for c in range(NC)], axis=1)
    return out.astype(np.float32)
